# revision 35
# baseline (speedup 1.0000x reference)
"""Trainium2 Bass kernel for nn_Net_89361089561102 (2-layer dense transformer,
NF4-quantized weights, cls head). Tensor-parallel over 8 NeuronCores.

Strategy:
 - Host: unpack NF4 weights -> bf16, pre-transpose to [K, M] layout, shard
   TP-style (qkv/gate_up by output dim == heads/ff, o/down by output dim),
   embedding gather, RoPE cos/sin tables, causal masks.
 - Device (per core, feature-major activations [H partitions, tokens free]):
   rmsnorm (PE ones-matmul partition reductions), qkv projections, RoPE,
   attention with transposed scores [tk, tq] (softmax denominator via PE),
   AllGather(ctx) -> o_proj (output-sharded) -> AllGather(x), gated MLP with
   AllGather(intermediate) -> down (output-sharded) -> AllGather(x).
 - Layer 2 (last layer): q/o/MLP evaluated only at the last token of each
   batch (the only positions that reach the output); k/v still full.
 - Final rmsnorm + cls head (Linear-ReLU-LayerNorm-Linear) computed
   redundantly on every core for the 2 last tokens.
"""

import math
from contextlib import ExitStack
from dataclasses import dataclass

import numpy as np
import ml_dtypes

BF16 = ml_dtypes.bfloat16
EPS = 1e-5
BLK = 64
NF4 = np.array([
    -1.0, -0.6961928009986877, -0.5250730514526367, -0.39491748809814453,
    -0.28444138169288635, -0.18477343022823334, -0.09105003625154495, 0.0,
    0.07958029955625534, 0.16093020141124725, 0.24611230194568634,
    0.33791524171829224, 0.44070982933044434, 0.5626170039176941,
    0.7229568362236023, 1.0], dtype=np.float32)


@dataclass(frozen=True)
class Cfg:
    H: int
    NH: int
    HD: int
    FF: int
    B: int
    S: int
    L: int
    NC: int
    CLS: int = 768
    NCLS: int = 2
    P: int = 128

    @property
    def T(self):
        return self.B * self.S

    @property
    def KT(self):
        return self.H // self.P

    @property
    def KTF(self):
        return self.FF // self.P

    @property
    def HPC(self):  # heads per core
        return self.NH // self.NC

    @property
    def DR(self):  # q/k/v rows per core
        return self.HPC * self.HD

    @property
    def OR(self):  # o/down output rows per core
        return self.H // self.NC

    @property
    def OT(self):
        return self.OR // self.P

    @property
    def FPC(self):  # ff rows per core
        return self.FF // self.NC

    @property
    def FT(self):
        return self.FPC // self.P

    @property
    def SP(self):  # seq tiles per batch
        return self.S // self.P

    @property
    def TP_(self):  # token tiles total
        return self.T // self.P

    @property
    def CT(self):
        return self.CLS // self.P

    def check(self):
        assert self.H % self.P == 0 and self.FF % self.P == 0
        assert self.S % self.P == 0 and self.S <= 512
        assert self.NH % self.NC == 0 and self.H % self.NC == 0
        assert self.FF % self.NC == 0
        assert self.OR % self.P == 0 and self.FPC % self.P == 0
        assert self.HD <= self.P and self.HD % 2 == 0
        assert self.HPC * self.B <= 8  # q/k psum groups
        assert self.TP_ <= 8           # v psum groups
        assert self.OT * self.B <= 8   # o/down psum groups
        assert self.CLS % self.P == 0


FULL_CFG = Cfg(H=3072, NH=32, HD=96, FF=8192, B=2, S=512, L=2, NC=8)


# ----------------------------------------------------------------------------
# host-side prep
# ----------------------------------------------------------------------------

def dequant_np(packed, absmax, out_f, in_f):
    shifts = (np.arange(8, dtype=np.int32) * 4)
    codes = ((packed[:, None] >> shifts) & 0xF).reshape(-1)
    w = (NF4[codes].reshape(-1, BLK) * absmax[:, None].astype(np.float32))
    return w.reshape(out_f, in_f)


def _wt3(w_t, P):
    """[K, M] fp32 -> [K//P, P, M] bf16 contiguous."""
    K, M = w_t.shape
    return np.ascontiguousarray(w_t.reshape(K // P, P, M).astype(BF16))


def host_prep(cfg: Cfg, inputs):
    """Full inputs -> list of per-core input maps."""
    c = cfg
    P = c.P
    x = inputs["embed"][inputs["input_ids"]]          # [B, S, H] fp32
    x0f = np.ascontiguousarray(x.reshape(c.T, c.H).T.astype(np.float32))
    x0 = np.ascontiguousarray(x0f.astype(BF16))        # [H, T] bf16

    # rope tables
    inv = 1.0 / (10000.0 ** (np.arange(0, c.HD, 2, dtype=np.float32) / c.HD))
    f = np.outer(np.arange(c.S, dtype=np.float32), inv)
    emb = np.concatenate([f, f], -1)                   # [S, HD]
    sgn = np.concatenate([-np.ones(c.HD // 2, np.float32),
                          np.ones(c.HD // 2, np.float32)])
    cosT = np.tile(np.cos(emb).T, (1, c.B))            # [HD, T]
    sinT = np.tile(np.sin(emb).T * sgn[:, None], (1, c.B))
    last = np.array([b * c.S + c.S - 1 for b in range(c.B)])
    cosT2 = np.ascontiguousarray(cosT[:, last].astype(np.float32))
    sinT2 = np.ascontiguousarray(sinT[:, last].astype(np.float32))
    cosT = np.ascontiguousarray(cosT.astype(BF16))
    sinT = np.ascontiguousarray(sinT.astype(BF16))

    am = (inputs["attention_mask"] != 0)               # [B, S]
    tk = np.arange(c.S)
    m1 = np.zeros((c.B, c.SP, P, c.S), np.float32)
    for b in range(c.B):
        for t in range(c.SP):
            rows = tk[t * P:(t + 1) * P]
            m1[b, t] = ((rows[:, None] <= tk[None, :]) & am[b, rows][:, None])
    m1 = m1.astype(BF16)
    am2 = np.zeros((c.B, P, c.SP), np.float32)
    for b in range(c.B):
        am2[b] = am[b].reshape(c.SP, P).T
    am2 = am2.astype(BF16)

    # layernorm weights [5, P, KT]
    lnw = np.zeros((2 * c.L + 1, P, c.KT), np.float32)
    for l in range(c.L):
        lnw[2 * l] = inputs["ln1_w"][l].reshape(c.KT, P).T
        lnw[2 * l + 1] = inputs["ln2_w"][l].reshape(c.KT, P).T
    lnw[2 * c.L] = inputs["final_ln_w"].reshape(c.KT, P).T

    # cls head
    w1t = _wt3(inputs["w1"].astype(np.float32).T, P)       # [KT, P, CLS]
    b1c = np.ascontiguousarray(
        inputs["b1"].reshape(c.CT, P).T.astype(np.float32))
    gcol = np.ascontiguousarray(
        inputs["ln_g"].reshape(c.CT, P).T.astype(np.float32))
    bcol = np.ascontiguousarray(
        inputs["ln_b"].reshape(c.CT, P).T.astype(np.float32))
    w2t = np.ascontiguousarray(
        inputs["w2"].astype(np.float32).T.reshape(c.CT, P, c.NCLS).astype(BF16))
    b2c = np.ascontiguousarray(
        inputs["b2"].reshape(c.NCLS, 1).astype(np.float32))
    eye2 = np.eye(c.B, dtype=np.float32)
    eye2b = eye2.astype(BF16)

    shared = dict(x0=x0, cosT=cosT, sinT=sinT, cosT2=cosT2, sinT2=sinT2,
                  m1=m1, am2=am2, lnw=lnw, w1t=w1t, b1c=b1c, gcol=gcol,
                  bcol=bcol, w2t=w2t, b2c=b2c, eye2=eye2, eye2b=eye2b)

    # per-layer dequantized weights (full) then shard
    per_layer = []
    for l in range(c.L):
        wqkv = dequant_np(inputs["qkv_packed"][l], inputs["qkv_absmax"][l],
                          3 * c.H, c.H)
        wo = dequant_np(inputs["o_packed"][l], inputs["o_absmax"][l],
                        c.H, c.H)
        wgu = dequant_np(inputs["gu_packed"][l], inputs["gu_absmax"][l],
                         2 * c.FF, c.H)
        wd = dequant_np(inputs["down_packed"][l], inputs["down_absmax"][l],
                        c.H, c.FF)
        per_layer.append((wqkv, wo, wgu, wd))

    in_maps = []
    for core in range(c.NC):
        m = dict(shared)
        m["x0r"] = np.ascontiguousarray(
            x0f[core * c.OR:(core + 1) * c.OR, :])
        for l in range(c.L):
            wqkv, wo, wgu, wd = per_layer[l]
            d0 = core * c.DR
            m[f"wq{l}"] = _wt3(wqkv[d0:d0 + c.DR, :].T, P)
            m[f"wk{l}"] = _wt3(wqkv[c.H + d0:c.H + d0 + c.DR, :].T, P)
            m[f"wv{l}"] = _wt3(wqkv[2 * c.H + d0:2 * c.H + d0 + c.DR, :].T, P)
            o0 = core * c.OR
            m[f"wo{l}"] = _wt3(wo[o0:o0 + c.OR, :].T, P)
            g0 = core * c.FPC
            m[f"wg{l}"] = _wt3(wgu[g0:g0 + c.FPC, :].T, P)
            m[f"wu{l}"] = _wt3(wgu[c.FF + g0:c.FF + g0 + c.FPC, :].T, P)
            m[f"wd{l}"] = _wt3(wd[o0:o0 + c.OR, :].T, P)
        # last-layer slim path: o/down sharded by INPUT dim (this core's
        # ctx heads / ff rows), full output rows -> partial sums + AllReduce
        wqkv, wo, wgu, wd = per_layer[c.L - 1]
        d0 = core * c.DR
        wos = np.zeros((c.HPC, P, c.H), np.float32)
        for h in range(c.HPC):
            wos[h, 0:c.HD, :] = wo[:, d0 + h * c.HD:d0 + (h + 1) * c.HD].T
        m["wos"] = np.ascontiguousarray(wos.astype(BF16))
        f0 = core * c.FPC
        m["wds"] = np.ascontiguousarray(
            wd[:, f0:f0 + c.FPC].T.reshape(c.FT, P, c.H).astype(BF16))
        in_maps.append(m)
    return in_maps


# ----------------------------------------------------------------------------
# device kernel
# ----------------------------------------------------------------------------

def build_nc(cfg: Cfg):
    import concourse.bass as bass
    import concourse.mybir as mybir
    import concourse.tile as tile
    from concourse import bacc

    c = cfg
    c.check()
    P = c.P
    f32 = mybir.dt.float32
    bf16 = mybir.dt.bfloat16
    AF = mybir.ActivationFunctionType
    OP = mybir.AluOpType

    nc = bacc.Bacc("TRN2", target_bir_lowering=False, debug=False,
                   enable_asserts=False, num_devices=c.NC)
    RG = [list(range(c.NC))]
    SHARED = "Shared" if c.NC > 4 else "Local"

    def din(name, shape, dt):
        return nc.dram_tensor(name, list(shape), dt, kind="ExternalInput").ap()

    x0 = din("x0", [c.H, c.T], bf16)
    x0r = din("x0r", [c.OR, c.T], f32)
    cosT = din("cosT", [c.HD, c.T], bf16)
    sinT = din("sinT", [c.HD, c.T], bf16)
    cosT2 = din("cosT2", [c.HD, c.B], f32)
    sinT2 = din("sinT2", [c.HD, c.B], f32)
    m1 = din("m1", [c.B, c.SP, P, c.S], bf16)
    am2 = din("am2", [c.B, P, c.SP], bf16)
    lnw_d = din("lnw", [2 * c.L + 1, P, c.KT], f32)
    w1t = din("w1t", [c.KT, P, c.CLS], bf16)
    b1c = din("b1c", [P, c.CT], f32)
    gcol = din("gcol", [P, c.CT], f32)
    bcol = din("bcol", [P, c.CT], f32)
    w2t = din("w2t", [c.CT, P, c.NCLS], bf16)
    b2c = din("b2c", [c.NCLS, 1], f32)
    eye2_d = din("eye2", [c.B, c.B], f32)
    eye2b_d = din("eye2b", [c.B, c.B], bf16)
    wos_d = din("wos", [c.HPC, P, c.H], bf16)
    wds_d = din("wds", [c.FT, P, c.H], bf16)
    wq = [din(f"wq{l}", [c.KT, P, c.DR], bf16) for l in range(c.L)]
    wk = [din(f"wk{l}", [c.KT, P, c.DR], bf16) for l in range(c.L)]
    wv = [din(f"wv{l}", [c.KT, P, c.DR], bf16) for l in range(c.L)]
    wo = [din(f"wo{l}", [c.KT, P, c.OR], bf16) for l in range(c.L)]
    wg = [din(f"wg{l}", [c.KT, P, c.FPC], bf16) for l in range(c.L)]
    wu = [din(f"wu{l}", [c.KT, P, c.FPC], bf16) for l in range(c.L)]
    wd = [din(f"wd{l}", [c.KTF, P, c.OR], bf16) for l in range(c.L)]
    out_d = nc.dram_tensor("logits_out", [c.NCLS, c.B], f32,
                           kind="ExternalOutput").ap()

    isqrt_hd = 1.0 / math.sqrt(c.HD)

    def lastcols(ap2d):
        """[P, T] AP -> [P, B] AP selecting the last token of each batch."""
        return ap2d.rearrange("p (b s) -> p b s", s=c.S)[:, :, c.S - 1]

    with tile.TileContext(nc) as tc, ExitStack() as ctx:
        const = ctx.enter_context(tc.tile_pool(name="const", bufs=1))
        persist = ctx.enter_context(tc.tile_pool(name="persist", bufs=1))
        wpool = ctx.enter_context(tc.tile_pool(name="wpool", bufs=3))
        xpool = ctx.enter_context(tc.tile_pool(name="xpool", bufs=3))
        spool = ctx.enter_context(tc.tile_pool(name="spool", bufs=2))
        ppool = ctx.enter_context(tc.tile_pool(name="ppool", bufs=3))
        rpool = ctx.enter_context(tc.tile_pool(name="rpool", bufs=1))
        psum = ctx.enter_context(tc.tile_pool(name="psum", bufs=8,
                                              space="PSUM"))
        dram = ctx.enter_context(tc.tile_pool(name="dram", bufs=1,
                                              space="DRAM"))

        # ---- constants in SBUF ----
        ones_c32 = const.tile([P, 1], f32, tag="ones_c32")
        nc.vector.memset(ones_c32[:], 1.0)
        ones_cbf = const.tile([P, 1], bf16, tag="ones_cbf")
        nc.vector.memset(ones_cbf[:], 1.0)
        ones_r32 = const.tile([1, P], f32, tag="ones_r32")
        nc.vector.memset(ones_r32[:], 1.0)
        eps_col = const.tile([P, 1], f32, tag="eps_col")
        nc.vector.memset(eps_col[:], EPS)
        cos_sb = const.tile([c.HD, c.T], bf16, tag="cos_sb")
        nc.sync.dma_start(out=cos_sb[:], in_=cosT)
        sin_sb = const.tile([c.HD, c.T], bf16, tag="sin_sb")
        nc.sync.dma_start(out=sin_sb[:], in_=sinT)
        cos2_sb = const.tile([c.HD, c.B], f32, tag="cos2_sb")
        nc.sync.dma_start(out=cos2_sb[:], in_=cosT2)
        sin2_sb = const.tile([c.HD, c.B], f32, tag="sin2_sb")
        nc.sync.dma_start(out=sin2_sb[:], in_=sinT2)
        mask_sb = const.tile([P, c.B * c.SP, c.S], bf16, tag="mask_sb")
        for b in range(c.B):
            for t in range(c.SP):
                nc.sync.dma_start(out=mask_sb[:, b * c.SP + t, :],
                                  in_=m1[b, t])
        am2_sb = const.tile([P, c.B, c.SP], bf16, tag="am2_sb")
        for b in range(c.B):
            nc.sync.dma_start(out=am2_sb[:, b, :], in_=am2[b])
        lnw_sb = const.tile([P, 2 * c.L + 1, c.KT], f32, tag="lnw_sb")
        for n in range(2 * c.L + 1):
            nc.sync.dma_start(out=lnw_sb[:, n, :], in_=lnw_d[n])
        b1_sb = const.tile([P, c.CT], f32, tag="b1_sb")
        nc.sync.dma_start(out=b1_sb[:], in_=b1c)
        g_sb_c = const.tile([P, c.CT], f32, tag="g_sb_c")
        nc.sync.dma_start(out=g_sb_c[:], in_=gcol)
        bcol_sb = const.tile([P, c.CT], f32, tag="bcol_sb")
        nc.sync.dma_start(out=bcol_sb[:], in_=bcol)
        b2_sb = const.tile([c.NCLS, 1], f32, tag="b2_sb")
        nc.sync.dma_start(out=b2_sb[:], in_=b2c)
        eye2_sb = const.tile([c.B, c.B], f32, tag="eye2_sb")
        nc.sync.dma_start(out=eye2_sb[:], in_=eye2_d)
        eye2b_sb = const.tile([c.B, c.B], bf16, tag="eye2b_sb")
        nc.sync.dma_start(out=eye2b_sb[:], in_=eye2b_d)

        # ---- collective warm-up: absorb channel-establish cost under
        # the first compute phase (first real AG otherwise pays ~200us) ----
        wu_sb = const.tile([P, 512], f32, tag="wu_sb")
        nc.vector.memset(wu_sb[:], 0.0)
        wu_in = dram.tile([P, 512], f32, tag="wu_in", name="wu_in")
        wu_out = dram.tile([P * c.NC, 512], f32, addr_space=SHARED,
                           tag="wu_out", name="wu_out")
        nc.sync.dma_start(out=wu_in[:], in_=wu_sb[:])
        nc.gpsimd.collective_compute(
            "AllGather", OP.bypass, replica_groups=RG,
            ins=[wu_in[:]], outs=[wu_out[:]])

        # ---- persistent activation state ----
        xn = persist.tile([P, c.KT, c.T], bf16, tag="xn")       # normalized x (bf16)
        xrows = persist.tile([P, c.OT, c.T], f32, tag="xrows")     # this core's rows of x
        for ot in range(c.OT):
            nc.sync.dma_start(out=xrows[:, ot, :],
                              in_=x0r[ot * P:(ot + 1) * P, :])

        # ---------- helpers ----------
        def emit_norm(src_ap, lnidx, dst, ncols, chunks):
            """rmsnorm of src [H, ncols] (bf16) -> dst [P, KT, ncols] (bf16).
            Chunk-wise so working tiles stay <= 512 cols wide."""
            ss = [psum.tile([1, cw], f32, tag="ps", name=f"ssps{lnidx}_{ci}")
                  for ci, (c0, cw) in enumerate(chunks)]
            for kt in range(c.KT):
                for ci, (c0, cw) in enumerate(chunks):
                    xf = xpool.tile([P, cw], bf16, tag="xf", name="xf",
                                    bufs=2)
                    nc.sync.dma_start(
                        out=xf[:], in_=src_ap[kt * P:(kt + 1) * P,
                                              c0:c0 + cw])
                    nc.vector.tensor_copy(dst[:, kt, c0:c0 + cw], xf[:])
                    sq = xpool.tile([P, cw], bf16, tag="sq", name="sq",
                                    bufs=2)
                    nc.vector.tensor_mul(sq[:], xf[:], xf[:])
                    nc.tensor.matmul(ss[ci][:], ones_cbf[:], sq[:],
                                     start=(kt == 0), stop=(kt == c.KT - 1))
            for ci, (c0, cw) in enumerate(chunks):
                lt = spool.tile([1, cw], f32, tag="lt", name="lt")
                nc.scalar.activation(lt[:], ss[ci][:], AF.Ln,
                                     bias=eps_col[0:1, :], scale=1.0 / c.H)
                rt = spool.tile([1, cw], f32, tag="rt", name="rt")
                nc.scalar.activation(rt[:], lt[:], AF.Exp, scale=-0.5)
                bb = psum.tile([P, cw], f32, tag="ps", name="bbps")
                nc.tensor.matmul(bb[:], ones_r32[:], rt[:],
                                 start=True, stop=True)
                bc = spool.tile([P, cw], f32, tag="bc", name="bc", bufs=2)
                nc.scalar.copy(bc[:], bb[:])
                for kt in range(c.KT):
                    nc.vector.scalar_tensor_tensor(
                        dst[:, kt, c0:c0 + cw], dst[:, kt, c0:c0 + cw],
                        lnw_sb[:, lnidx, kt:kt + 1], bc[:],
                        OP.mult, OP.mult)

        def emit_rstd_bcast(ss_aps, lnidx, ncols, chunks):
            """ss_aps: per-chunk [1, cw] APs of full-H sum-of-squares.
            Returns bc_sb [P, ncols] f32 with rsqrt(mean+eps) per token."""
            bc = spool.tile([P, ncols], f32, tag="bc", name="bc", bufs=2)
            for ci, (c0, cw) in enumerate(chunks):
                lt = spool.tile([1, cw], f32, tag="lt", name="lt")
                nc.scalar.activation(lt[:], ss_aps[ci], AF.Ln,
                                     bias=eps_col[0:1, :], scale=1.0 / c.H)
                rt = spool.tile([1, cw], f32, tag="rt", name="rt")
                nc.scalar.activation(rt[:], lt[:], AF.Exp, scale=-0.5)
                bb = psum.tile([P, cw], f32, tag="ps", name="bbps")
                nc.tensor.matmul(bb[:], ones_r32[:], rt[:],
                                 start=True, stop=True)
                nc.scalar.copy(bc[:, c0:c0 + cw], bb[:])
            return bc

        def emit_sumsq_ar(x0_, cw, tag):
            """Partial sum-of-squares of this core's fp32 x rows over token
            columns [x0_, x0_+cw), then a tiny AllReduce (issued before the
            x AllGather so the norm scale is ready when x streams back)."""
            ssq = psum.tile([1, cw], f32, tag="ps", name=f"ssA{tag}")
            for ot in range(c.OT):
                sqr = xpool.tile([P, cw], bf16, tag="sqr", name="sqr",
                                 bufs=2)
                nc.vector.tensor_mul(sqr[:], xrows[:, ot, x0_:x0_ + cw],
                                     xrows[:, ot, x0_:x0_ + cw])
                nc.tensor.matmul(ssq[:], ones_cbf[:], sqr[:],
                                 start=(ot == 0), stop=(ot == c.OT - 1))
            srow = spool.tile([1, cw], f32, tag="srow", name="srow", bufs=2)
            nc.scalar.copy(srow[:], ssq[:])
            ssb = dram.tile([1, cw], f32, tag=f"ssb{tag}",
                            name=f"ssb{tag}")
            ssg = dram.tile([1, cw], f32, addr_space=SHARED,
                            tag=f"ssg{tag}", name=f"ssg{tag}")
            nc.sync.dma_start(out=ssb[:], in_=srow[:])
            nc.gpsimd.collective_compute(
                "AllReduce", OP.add, replica_groups=RG,
                ins=[ssb[:]], outs=[ssg[:]])
            return ssg

        def emit_norm_post(ssg, src_ap, lnidx, dst, dst_c0, ncols, chunks):
            """normalize src [H, ncols] into dst[:, kt, dst_c0:dst_c0+ncols]"""
            sst = spool.tile([1, ncols], f32, tag="sst", name="sst", bufs=2)
            nc.sync.dma_start(out=sst[:], in_=ssg[:])
            bc = emit_rstd_bcast(
                [sst[:, c0:c0 + cw] for (c0, cw) in chunks],
                lnidx, ncols, chunks)
            for kt in range(c.KT):
                xf = xpool.tile([P, ncols], bf16, tag="xf", name="xfa",
                                bufs=2)
                nc.sync.dma_start(out=xf[:],
                                  in_=src_ap[kt * P:(kt + 1) * P, :])
                nc.vector.scalar_tensor_tensor(
                    dst[:, kt, dst_c0:dst_c0 + ncols], xf[:],
                    lnw_sb[:, lnidx, kt:kt + 1],
                    bc[:], OP.mult, OP.mult)

        def emit_norm_slim(xs, lnidx, dst3):
            """rmsnorm of an SBUF [P, KT, B] tile: local sumsq."""
            sq = spool.tile([P, c.KT, c.B], f32, tag="sq_slim",
                            name="sq_slim")
            nc.vector.tensor_mul(sq[:], xs[:], xs[:])
            sp_ = psum.tile([1, c.KT * c.B], f32, tag="ps", name="spslim")
            nc.tensor.matmul(sp_[:], ones_c32[:],
                             sq[:].rearrange("p kt b -> p (kt b)"),
                             start=True, stop=True)
            ss2 = spool.tile([1, c.B], f32, tag="ss2", name="ss2")
            nc.vector.tensor_reduce(
                ss2[:], sp_[:].rearrange("o (kt b) -> o b kt", b=c.B),
                mybir.AxisListType.X, OP.add)
            bc = emit_rstd_bcast([ss2[:]], lnidx, c.B, [(0, c.B)])
            tmp = spool.tile([P, c.KT, c.B], f32, tag="tmp_slim",
                             name="tmp_slim")
            nc.vector.tensor_tensor(
                tmp[:], xs[:],
                lnw_sb[:, lnidx, :].unsqueeze(2).broadcast_to(
                    (P, c.KT, c.B)), OP.mult)
            nc.vector.tensor_tensor(
                dst3[:], tmp[:],
                bc[:].unsqueeze(1).broadcast_to((P, c.KT, c.B)), OP.mult)

        def kouter_pass(KK, wsrc, wcols, groups, rhs_fn, rhs_load=None,
                        name="kp"):
            """Generic contraction pass: loop k tiles (batched weight DMA),
            stream weights, accumulate len(groups) psum tiles.
            groups: list of (lhs_c0, lhs_cw, out_n, rhs_key)."""
            ps = [psum.tile([cw, n], f32, tag="ps", name=f"{name}{gi}")
                  for gi, (c0, cw, n, rk) in enumerate(groups)]
            G = max(1, min(4, 2048 // wcols))
            for k0 in range(0, KK, G):
                g_n = min(G, KK - k0)
                wt = wpool.tile([P, G, 2048 // G if wcols > 2048 // G else wcols],
                                bf16, tag="wt", name=f"{name}w", bufs=2)
                nc.sync.dma_start(
                    out=wt[:, 0:g_n, 0:wcols],
                    in_=wsrc(k0, g_n).rearrange("g p m -> p g m"))
                for g in range(g_n):
                    kt = k0 + g
                    rl = rhs_load(kt) if rhs_load is not None else None
                    for gi, (c0, cw, n, rk) in enumerate(groups):
                        nc.tensor.matmul(ps[gi][:], wt[:, g, c0:c0 + cw],
                                         rhs_fn(kt, rk, rl),
                                         start=(kt == 0), stop=(kt == KK - 1))
            return ps

        def emit_rope(src_ps, qr_dst, cos_ap, sin_ap, ncols):
            """rope: qr_dst = src*cos + swap_half(src)*sin_signed."""
            h2 = c.HD // 2
            qs = rpool.tile([c.HD, ncols], bf16, tag="qs", name="qs")
            nc.vector.tensor_copy(qs[:], src_ps[:])
            rot = rpool.tile([c.HD, ncols], bf16, tag="rot", name="rot")
            nc.sync.dma_start(out=rot[0:h2, :], in_=qs[h2:c.HD, :])
            nc.sync.dma_start(out=rot[h2:c.HD, :], in_=qs[0:h2, :])
            nc.vector.tensor_mul(qs[:], qs[:], cos_ap)
            nc.vector.tensor_mul(rot[:], rot[:], sin_ap)
            nc.vector.tensor_add(qr_dst, qs[:], rot[:])

        # ================= transformer layers =================
        x_src = x0
        ln1_ssg = None
        for l in range(c.L):
            slim = (l == c.L - 1)
            ncol2 = c.B if slim else c.T
            full_chunks = [(b * c.S, c.S) for b in range(c.B)]

            # resident q/k/v weights for this layer (DMA'd early, big xfers)
            wq_sb = None
            if not slim:
                wq_sb = wpool.tile([P, c.KT, c.DR], bf16, tag="wq_sb",
                                   name=f"wq_sb{l}", bufs=1)
                nc.sync.dma_start(out=wq_sb[:],
                                  in_=wq[l].rearrange("kt p m -> p kt m"))
            wk_sb = wpool.tile([P, c.KT, c.DR], bf16, tag="wk_sb",
                               name=f"wk_sb{l}", bufs=1)
            nc.sync.dma_start(out=wk_sb[:],
                              in_=wk[l].rearrange("kt p m -> p kt m"))

            # ---- ln1 + qkv ----
            if ln1_ssg is None:
                emit_norm(x_src, 2 * l, xn, c.T, full_chunks)
            else:
                # per-batch: x_src is a list of per-batch [H, S] gathers
                for b in range(c.B):
                    emit_norm_post(ln1_ssg[b], x_src[b], 2 * l, xn,
                                   b * c.S, c.S, [(0, c.S)])

            q_rot = persist.tile([c.HD, c.HPC, ncol2], bf16, tag="qrot",
                                 name=f"qrot{l}")
            k_rot = persist.tile([c.HD, c.HPC, c.T], bf16, tag="krot",
                                 name=f"krot{l}")
            v_sb = persist.tile([P, c.TP_, c.DR], bf16, tag="vsb",
                                name=f"vsb{l}")

            # q pass (slim: only last token of each batch)
            if slim:
                qg = [(h * c.HD, c.HD, c.B, 0) for h in range(c.HPC)]
                qrhs = lambda kt, rk, rl: lastcols(xn[:, kt, :])
                qps = kouter_pass(c.KT, lambda k0, n: wq[l][k0:k0 + n], c.DR,
                                  qg, qrhs, name="qp")
                for gi, (c0, cw, n, rk) in enumerate(qg):
                    h = c0 // c.HD
                    emit_rope(qps[gi], q_rot[:, h, :], cos2_sb[:], sin2_sb[:],
                              c.B)
            else:
                # head-outer, K-contiguous: rope of head i overlaps matmuls
                # of head i+1
                for h in range(c.HPC):
                    for b in range(c.B):
                        qp = psum.tile([c.HD, c.S], f32, tag="ps",
                                       name="qhps")
                        for kt in range(c.KT):
                            nc.tensor.matmul(
                                qp[:], wq_sb[:, kt, h * c.HD:(h + 1) * c.HD],
                                xn[:, kt, b * c.S:(b + 1) * c.S],
                                start=(kt == 0), stop=(kt == c.KT - 1))
                        emit_rope(qp, q_rot[:, h, b * c.S:(b + 1) * c.S],
                                  cos_sb[:, b * c.S:(b + 1) * c.S],
                                  sin_sb[:, b * c.S:(b + 1) * c.S], c.S)

            # k pass (always full), head-outer
            for h in range(c.HPC):
                for b in range(c.B):
                    kp = psum.tile([c.HD, c.S], f32, tag="ps",
                                   name="khps")
                    for kt in range(c.KT):
                        nc.tensor.matmul(
                            kp[:], wk_sb[:, kt, h * c.HD:(h + 1) * c.HD],
                            xn[:, kt, b * c.S:(b + 1) * c.S],
                            start=(kt == 0), stop=(kt == c.KT - 1))
                    emit_rope(kp, k_rot[:, h, b * c.S:(b + 1) * c.S],
                              cos_sb[:, b * c.S:(b + 1) * c.S],
                              sin_sb[:, b * c.S:(b + 1) * c.S], c.S)

            # v pass (token-major): psum groups per token tile
            vps = [psum.tile([P, c.DR], f32, tag="ps", name=f"vp{tt}")
                   for tt in range(c.TP_)]
            for kt in range(c.KT):
                wt = wpool.tile([P, c.DR], bf16, tag="wt", name="vw", bufs=2)
                nc.sync.dma_start(out=wt[:], in_=wv[l][kt])
                for tt in range(c.TP_):
                    nc.tensor.matmul(vps[tt][:],
                                     xn[:, kt, tt * P:(tt + 1) * P], wt[:],
                                     start=(kt == 0), stop=(kt == c.KT - 1))
            for tt in range(c.TP_):
                nc.scalar.copy(v_sb[:, tt, :], vps[tt][:])

            # ---- attention ----
            if slim:
                # ctx stays local in SBUF (o is input-sharded; partitions
                # HD..P zero so padded o rows contribute nothing)
                ctx_sb = persist.tile([P, c.HPC, c.B], bf16, tag="ctx_sb",
                                      name="ctx_sb")
                nc.vector.memset(ctx_sb[:], 0.0)
                ctxbs, ctxgs = [], []
            else:
                ctxbs = [dram.tile([c.DR, c.S], bf16, tag=f"ctxb{l}_{b}",
                                   name=f"ctxb{l}_{b}") for b in range(c.B)]
                ctxgs = [dram.tile([c.H, c.S], bf16, addr_space=SHARED,
                                   tag=f"ctxg{l}_{b}", name=f"ctxg{l}_{b}")
                         for b in range(c.B)]
            for b in range(c.B):
                for h in range(c.HPC):
                    if not slim:
                        den = psum.tile([1, c.S], f32, tag="ps", name="den")
                        cps = psum.tile([c.HD, c.S], f32, tag="ps", name="cps")
                        for t in range(c.SP):
                            n0 = t * P  # causal: tile t only sees tq >= t*P
                            sps = psum.tile([P, c.S], f32, tag="ps",
                                            name="sps")
                            nc.tensor.matmul(
                                sps[:, n0:],
                                k_rot[:, h, b * c.S + t * P:
                                      b * c.S + (t + 1) * P],
                                q_rot[:, h, b * c.S + n0:(b + 1) * c.S],
                                start=True, stop=True)
                            pt = ppool.tile([P, c.S], bf16, tag="pt",
                                            name="pt")
                            nc.scalar.activation(pt[:, n0:], sps[:, n0:],
                                                 AF.Exp, scale=isqrt_hd)
                            nc.vector.tensor_mul(
                                pt[:, n0:], pt[:, n0:],
                                mask_sb[:, b * c.SP + t, n0:])
                            nc.tensor.matmul(den[:, n0:], ones_cbf[:],
                                             pt[:, n0:],
                                             start=(t == 0),
                                             stop=(t == c.SP - 1))
                            nc.tensor.matmul(
                                cps[:, n0:],
                                v_sb[:, b * c.SP + t,
                                     h * c.HD:(h + 1) * c.HD],
                                pt[:, n0:],
                                start=(t == 0), stop=(t == c.SP - 1))
                        # 1/den via Ln+Exp on Scalar (DVE reciprocal on a
                        # 1-partition tile is ~3.4us; this is ~1.3us)
                        lt = spool.tile([1, c.S], f32, tag="dr", name="dln")
                        nc.scalar.activation(lt[:], den[:], AF.Ln)
                        dr = spool.tile([1, c.S], f32, tag="dr", name="dr")
                        nc.scalar.activation(dr[:], lt[:], AF.Exp, scale=-1.0)
                        bb = psum.tile([c.HD, c.S], f32, tag="ps", name="bb")
                        nc.tensor.matmul(bb[:], ones_r32[:, 0:c.HD], dr[:],
                                         start=True, stop=True)
                        bsb = spool.tile([c.HD, c.S], f32, tag="csb",
                                         name="bsb", bufs=2)
                        nc.scalar.copy(bsb[:], bb[:])
                        csb = spool.tile([c.HD, c.S], bf16, tag="csb",
                                         name="csb", bufs=2)
                        nc.vector.tensor_mul(csb[:], cps[:], bsb[:])
                        nc.sync.dma_start(
                            out=ctxbs[b][h * c.HD:(h + 1) * c.HD, :],
                            in_=csb[:])
                    else:
                        sps = psum.tile([P, c.SP], f32, tag="ps", name="sps2")
                        for t in range(c.SP):
                            nc.tensor.matmul(
                                sps[:, t:t + 1],
                                k_rot[:, h, b * c.S + t * P:
                                      b * c.S + (t + 1) * P],
                                q_rot[:, h, b:b + 1],
                                start=True, stop=True)
                        pt = ppool.tile([P, c.SP], bf16, tag="pt2",
                                        name="pt2")
                        nc.scalar.activation(pt[:], sps[:], AF.Exp,
                                             scale=isqrt_hd)
                        nc.vector.tensor_mul(pt[:], pt[:], am2_sb[:, b, :])
                        dps = psum.tile([1, c.SP], f32, tag="ps", name="dps")
                        nc.tensor.matmul(dps[:], ones_cbf[:], pt[:],
                                         start=True, stop=True)
                        d1 = spool.tile([1, 1], f32, tag="d1", name="d1")
                        nc.vector.tensor_reduce(d1[:], dps[:],
                                                mybir.AxisListType.X, OP.add)
                        r1 = spool.tile([1, 1], f32, tag="r1", name="r1")
                        nc.vector.reciprocal(r1[:], d1[:])
                        cps = psum.tile([c.HD, 1], f32, tag="ps", name="cps2")
                        for t in range(c.SP):
                            nc.tensor.matmul(
                                cps[:],
                                v_sb[:, b * c.SP + t,
                                     h * c.HD:(h + 1) * c.HD],
                                pt[:, t:t + 1],
                                start=(t == 0), stop=(t == c.SP - 1))
                        bb = psum.tile([c.HD, 1], f32, tag="ps", name="bb2")
                        nc.tensor.matmul(bb[:], ones_r32[:, 0:c.HD], r1[:],
                                         start=True, stop=True)
                        bsb = spool.tile([c.HD, 1], f32, tag="bsb2",
                                         name="bsb2")
                        nc.vector.tensor_copy(bsb[:], bb[:])
                        nc.vector.tensor_mul(
                            ctx_sb[0:c.HD, h, b:b + 1], cps[:], bsb[:])
                if not slim:
                    nc.gpsimd.collective_compute(
                        "AllGather", OP.bypass, replica_groups=RG,
                        ins=[ctxbs[b][:]], outs=[ctxgs[b][:]])

            if slim:
                # 512-col chunks of the full H / FPC outputs
                hch = [(i * 512, min(512, c.H - i * 512))
                       for i in range((c.H + 511) // 512)]
                fch = [(i * 512, min(512, c.FPC - i * 512))
                       for i in range((c.FPC + 511) // 512)]
                HC = len(hch)

                def stream_w(ap2d, pcols, nm):
                    t = wpool.tile([P, pcols], bf16, tag="wt", name=nm,
                                   bufs=2)
                    nc.sync.dma_start(out=t[:], in_=ap2d)
                    return t

                def ar_round(ps_list, tag2):
                    """evacuate [B,512] psums -> DRAM, AllReduce, return
                    the gathered [B, H] f32 DRAM AP."""
                    arb = dram.tile([c.B, c.H], f32, tag=f"arb{tag2}",
                                    name=f"arb{tag2}")
                    arg_ = dram.tile([c.B, c.H], f32, addr_space=SHARED,
                                     tag=f"arg{tag2}", name=f"arg{tag2}")
                    for ci, (c0_, cw_) in enumerate(hch):
                        ev = spool.tile([c.B, cw_], f32, tag="sg1",
                                        name="ev", bufs=2)
                        nc.vector.tensor_copy(ev[:], ps_list[ci][:])
                        nc.sync.dma_start(
                            out=arb[:, c0_:c0_ + cw_], in_=ev[:])
                    nc.gpsimd.collective_compute(
                        "AllReduce", OP.add, replica_groups=RG,
                        ins=[arb[:]], outs=[arg_[:]])
                    return arg_

                def add_transposed(arg_, dst, residual_ap=None):
                    """dst[:, kt, :] (+)= transpose(arg_[:, ktP:(kt+1)P])"""
                    for kt in range(c.KT):
                        xc = spool.tile([c.B, P], f32, tag="xc", name="xc",
                                        bufs=2)
                        nc.sync.dma_start(
                            out=xc[:], in_=arg_[:, kt * P:(kt + 1) * P])
                        tp = psum.tile([P, c.B], f32, tag="ps", name="tp")
                        nc.tensor.transpose(tp[:], xc[:], eye2_sb[:])
                        if residual_ap is None:
                            nc.vector.tensor_add(dst[:, kt, :],
                                                 dst[:, kt, :], tp[:])
                        else:
                            nc.vector.tensor_add(dst[:, kt, :],
                                                 residual_ap[:, kt, :],
                                                 tp[:])

                # ---- o partial (input-sharded over this core's heads),
                # flipped: ctx [P,B] stationary, Wo^T chunks moving ----
                po = [psum.tile([c.B, cw_], f32, tag="ps",
                                name=f"po{ci}")
                      for ci, (c0_, cw_) in enumerate(hch)]
                for ci, (c0_, cw_) in enumerate(hch):
                    wt_ = wpool.tile([P, c.HPC, cw_], bf16, tag="wt",
                                     name="wos", bufs=2)
                    nc.sync.dma_start(
                        out=wt_[:],
                        in_=wos_d[:, :, c0_:c0_ + cw_].rearrange(
                            "h p m -> p h m"))
                    for h in range(c.HPC):
                        nc.tensor.matmul(
                            po[ci][:], ctx_sb[:, h, :], wt_[:, h, :],
                            start=(h == 0), stop=(h == c.HPC - 1))
                oarg = ar_round(po, "o")

                # residual base: last token column of each batch
                xlast = spool.tile([P, c.KT, c.B], bf16, tag="xlast",
                                   name="xlast")
                for b in range(c.B):
                    nc.sync.dma_start(
                        out=xlast[:, :, b:b + 1],
                        in_=x_src[b].rearrange(
                            "(kt p) s -> p kt s", p=P)[:, :, c.S - 1:c.S])
                xslim = persist.tile([P, c.KT, c.B], f32, tag="xslim",
                                     name="xslim")
                add_transposed(oarg, xslim, residual_ap=xlast)

                # ---- ln2 + gated MLP (flipped, output-sharded gu) ----
                xn2 = persist.tile([P, c.KT, c.B], bf16, tag="xn2",
                                   name="xn2")
                emit_norm_slim(xslim, 2 * l + 1, xn2)
                gt = [psum.tile([c.B, cw_], f32, tag="ps",
                                name=f"gt{i}")
                      for i, (c0_, cw_) in enumerate(fch)]
                ut = [psum.tile([c.B, cw_], f32, tag="ps",
                                name=f"ut{i}")
                      for i, (c0_, cw_) in enumerate(fch)]
                for kt in range(c.KT):
                    wg_t = stream_w(wg[l][kt], c.FPC, "wgs")
                    wu_t = stream_w(wu[l][kt], c.FPC, "wus")
                    for i, (c0_, cw_) in enumerate(fch):
                        nc.tensor.matmul(gt[i][:], xn2[:, kt, :],
                                         wg_t[:, c0_:c0_ + cw_],
                                         start=(kt == 0),
                                         stop=(kt == c.KT - 1))
                        nc.tensor.matmul(ut[i][:], xn2[:, kt, :],
                                         wu_t[:, c0_:c0_ + cw_],
                                         start=(kt == 0),
                                         stop=(kt == c.KT - 1))
                # silu(g)*u -> transpose chunks into [P, FT, B]
                int_c = persist.tile([P, c.FT, c.B], bf16, tag="int_c",
                                     name="int_c")
                for i, (c0_, cw_) in enumerate(fch):
                    sg = spool.tile([c.B, cw_], f32, tag="sg1", name="sg",
                                    bufs=2)
                    nc.scalar.activation(sg[:], gt[i][:], AF.Sigmoid)
                    nc.vector.tensor_mul(sg[:], gt[i][:], sg[:])
                    itc = spool.tile([c.B, cw_], bf16, tag="sg3", name="itc",
                                     bufs=2)
                    nc.vector.tensor_mul(itc[:], ut[i][:], sg[:])
                    for j in range(cw_ // P):
                        tp = psum.tile([P, c.B], bf16, tag="ps", name="tpi")
                        nc.tensor.transpose(tp[:], itc[:, j * P:(j + 1) * P],
                                            eye2b_sb[:])
                        nc.vector.tensor_copy(
                            int_c[:, (c0_ // P) + j, :], tp[:])

                # ---- down partial (input-sharded over this core's ff
                # rows) + AllReduce + residual ----
                pd = [psum.tile([c.B, cw_], f32, tag="ps",
                                name=f"pd{ci}")
                      for ci, (c0_, cw_) in enumerate(hch)]
                for ci, (c0_, cw_) in enumerate(hch):
                    for kt in range(c.FT):
                        wd_t = stream_w(
                            wds_d[kt][:, c0_:c0_ + cw_], cw_, "wdss")
                        nc.tensor.matmul(pd[ci][:], int_c[:, kt, :],
                                         wd_t[:],
                                         start=(kt == 0),
                                         stop=(kt == c.FT - 1))
                darg = ar_round(pd, "d")
                add_transposed(darg, xslim)
                continue

            # ======== non-slim: per-batch pipelined o / MLP / down ========
            # ---- o projection (+ residual into xrows), AG per batch ----
            xbo_b = [dram.tile([c.OR, c.S], bf16, tag=f"xbo{l}_{b}",
                               name=f"xbo{l}_{b}") for b in range(c.B)]
            xgo_b = [dram.tile([c.H, c.S], bf16, addr_space=SHARED,
                               tag=f"xgo{l}_{b}", name=f"xgo{l}_{b}")
                     for b in range(c.B)]
            ln2_ssg = []
            for b in range(c.B):
                og_b = [(ot * P, P, c.S, b) for ot in range(c.OT)]
                orhs = lambda kt, rk, rl: rl[:]

                def oload(kt, _b=b):
                    t = xpool.tile([P, c.S], bf16, tag="orhs",
                                   name="orhs", bufs=3)
                    nc.scalar.dma_start(
                        out=t[:],
                        in_=ctxgs[_b][kt * P:(kt + 1) * P, :])
                    return t
                ops_b = kouter_pass(c.KT, lambda k0, n: wo[l][k0:k0 + n],
                                    c.OR, og_b, orhs, rhs_load=oload,
                                    name=f"op{b}")
                for gi, (c0, cw, n, rk) in enumerate(og_b):
                    ot = c0 // P
                    xsl = xrows[:, ot, b * c.S:(b + 1) * c.S]
                    nc.vector.tensor_add(xsl, xsl, ops_b[gi][:])
                    st = xpool.tile([P, n], bf16, tag="xst", name="xst",
                                    bufs=2)
                    nc.scalar.copy(st[:], xsl)
                    nc.sync.dma_start(out=xbo_b[b][ot * P:(ot + 1) * P, :],
                                      in_=st[:])
                ln2_ssg.append(emit_sumsq_ar(b * c.S, c.S, tag=f"o{l}_{b}"))
                nc.gpsimd.collective_compute(
                    "AllGather", OP.bypass, replica_groups=RG,
                    ins=[xbo_b[b][:]], outs=[xgo_b[b][:]])

            # ---- ln2 + gated MLP, AG per batch; down chases the AGs ----
            gact = persist.tile([P, c.FT, c.S], bf16, tag="gact",
                                name=f"gact{l}")
            intb_b = [dram.tile([c.FPC, c.S], bf16, tag=f"intb{l}_{b}",
                                name=f"intb{l}_{b}") for b in range(c.B)]
            intg_b = [dram.tile([c.FF, c.S], bf16, addr_space=SHARED,
                                tag=f"intg{l}_{b}", name=f"intg{l}_{b}")
                      for b in range(c.B)]
            for b in range(c.B):
                emit_norm_post(ln2_ssg[b], xgo_b[b][:], 2 * l + 1, xn,
                               b * c.S, c.S, [(0, c.S)])
                for phase, wsrc3 in (("g", wg[l]), ("u", wu[l])):
                    gg = [(ot * P, P, c.S, ot) for ot in range(c.FT)]

                    def gsrc(k0, n, _w=wsrc3):
                        return _w[k0:k0 + n]
                    grhs = (lambda kt, rk, rl, _b=b:
                            xn[:, kt, _b * c.S:(_b + 1) * c.S])
                    gps = kouter_pass(c.KT, gsrc, c.FPC, gg, grhs,
                                      name=f"{phase}{b}")
                    for gi, (c0, cw, n, rk) in enumerate(gg):
                        ot = rk
                        if phase == "g":
                            sgt = xpool.tile([P, n], bf16, tag="sgt",
                                             name="sgt", bufs=2)
                            nc.scalar.activation(sgt[:], gps[gi][:],
                                                 AF.Sigmoid)
                            nc.vector.tensor_mul(gact[:, ot, :], gps[gi][:],
                                                 sgt[:])
                        else:
                            it = xpool.tile([P, n], bf16, tag="sgt",
                                            name="it", bufs=2)
                            nc.vector.tensor_mul(
                                it[:], gps[gi][:], gact[:, ot, :])
                            nc.sync.dma_start(
                                out=intb_b[b][ot * P:(ot + 1) * P, :],
                                in_=it[:])
                nc.gpsimd.collective_compute(
                    "AllGather", OP.bypass, replica_groups=RG,
                    ins=[intb_b[b][:]], outs=[intg_b[b][:]])

            # ---- down projection (+ residual), per batch ----
            xbd_b = [dram.tile([c.OR, c.S], bf16, tag=f"xbd{l}_{b}",
                               name=f"xbd{l}_{b}") for b in range(c.B)]
            xgd_b = [dram.tile([c.H, c.S], bf16, addr_space=SHARED,
                               tag=f"xgd{l}_{b}", name=f"xgd{l}_{b}")
                     for b in range(c.B)]
            ln1_ssg = []
            for b in range(c.B):
                dg_b = [(ot * P, P, c.S, b) for ot in range(c.OT)]
                drhs = lambda kt, rk, rl: rl[:]

                def dload(kt, _b=b):
                    t = xpool.tile([P, c.S], bf16, tag="orhs", name="drhs",
                                   bufs=3)
                    nc.scalar.dma_start(
                        out=t[:], in_=intg_b[_b][kt * P:(kt + 1) * P, :])
                    return t
                dps_b = kouter_pass(c.KTF, lambda k0, n: wd[l][k0:k0 + n],
                                    c.OR, dg_b, drhs, rhs_load=dload,
                                    name=f"dp{b}")
                for gi, (c0, cw, n, rk) in enumerate(dg_b):
                    ot = c0 // P
                    xsl = xrows[:, ot, b * c.S:(b + 1) * c.S]
                    nc.vector.tensor_add(xsl, xsl, dps_b[gi][:])
                    st = xpool.tile([P, n], bf16, tag="xst", name="xst2",
                                    bufs=2)
                    nc.scalar.copy(st[:], xsl)
                    nc.sync.dma_start(out=xbd_b[b][ot * P:(ot + 1) * P, :],
                                      in_=st[:])
                ln1_ssg.append(emit_sumsq_ar(b * c.S, c.S, tag=f"d{l}_{b}"))
                nc.gpsimd.collective_compute(
                    "AllGather", OP.bypass, replica_groups=RG,
                    ins=[xbd_b[b][:]], outs=[xgd_b[b][:]])
            x_src = xgd_b

        # ================= final norm + cls head =================
        xnf = persist.tile([P, c.KT, c.B], bf16, tag="xn2", name="xnf")
        emit_norm_slim(xslim, 2 * c.L, xnf)

        # flipped w1 pass: xnf [P,B] stationary, w1 chunks moving
        CC2 = (c.CLS + 511) // 512
        hts = [psum.tile([c.B, min(512, c.CLS - i * 512)], f32, tag="ps",
                         name=f"ht{i}") for i in range(CC2)]
        for kt in range(c.KT):
            wt = wpool.tile([P, c.CLS], bf16, tag="wt", name="w1w",
                            bufs=2)
            nc.sync.dma_start(out=wt[:], in_=w1t[kt])
            for i in range(CC2):
                cw_ = min(512, c.CLS - i * 512)
                nc.tensor.matmul(hts[i][:], xnf[:, kt, :],
                                 wt[:, i * 512:i * 512 + cw_],
                                 start=(kt == 0), stop=(kt == c.KT - 1))
        hflat = spool.tile([c.B, c.CLS], bf16, tag="sg1", name="hflat",
                           bufs=2)
        for i in range(CC2):
            cw_ = min(512, c.CLS - i * 512)
            nc.vector.tensor_copy(hflat[:, i * 512:i * 512 + cw_],
                                  hts[i][:])
        hps = []
        for ot in range(c.CT):
            tp = psum.tile([P, c.B], bf16, tag="ps", name=f"hps{ot}")
            nc.tensor.transpose(tp[:], hflat[:, ot * P:(ot + 1) * P],
                                eye2b_sb[:])
            hps.append(tp)
        h_sb = persist.tile([P, c.CT, c.B], bf16, tag="h_sb", name="h_sb")
        mn = psum.tile([1, c.B], f32, tag="ps", name="mn")
        ssq = psum.tile([1, c.B], f32, tag="ps", name="ssq")
        for ot in range(c.CT):
            nc.scalar.activation(h_sb[:, ot, :], hps[ot][:], AF.Relu,
                                 bias=b1_sb[:, ot:ot + 1])
            hq = spool.tile([P, c.B], f32, tag="hq", name="hq")
            nc.vector.tensor_mul(hq[:], h_sb[:, ot, :], h_sb[:, ot, :])
            nc.tensor.matmul(mn[:], ones_cbf[:], h_sb[:, ot, :],
                             start=(ot == 0), stop=(ot == c.CT - 1))
            nc.tensor.matmul(ssq[:], ones_c32[:], hq[:],
                             start=(ot == 0), stop=(ot == c.CT - 1))
        m_sb = spool.tile([1, c.B], f32, tag="m_sb", name="m_sb")
        nc.vector.tensor_scalar_mul(m_sb[:], mn[:], 1.0 / c.CLS)
        s_sb = spool.tile([1, c.B], f32, tag="s_sb", name="s_sb")
        nc.vector.tensor_scalar_mul(s_sb[:], ssq[:], 1.0 / c.CLS)
        msq = spool.tile([1, c.B], f32, tag="msq", name="msq")
        nc.vector.tensor_mul(msq[:], m_sb[:], m_sb[:])
        var = spool.tile([1, c.B], f32, tag="var", name="var")
        nc.vector.tensor_sub(var[:], s_sb[:], msq[:])
        lv = spool.tile([1, c.B], f32, tag="lv", name="lv")
        nc.scalar.activation(lv[:], var[:], AF.Ln, bias=eps_col[0:1, :])
        rstd = spool.tile([1, c.B], f32, tag="rstd", name="rstd")
        nc.scalar.activation(rstd[:], lv[:], AF.Exp, scale=-0.5)
        bmp = psum.tile([P, c.B], f32, tag="ps", name="bmp")
        nc.tensor.matmul(bmp[:], ones_r32[:], m_sb[:], start=True, stop=True)
        bm_sb = spool.tile([P, c.B], f32, tag="bm", name="bm")
        nc.vector.tensor_copy(bm_sb[:], bmp[:])
        brp = psum.tile([P, c.B], f32, tag="ps", name="brp")
        nc.tensor.matmul(brp[:], ones_r32[:], rstd[:], start=True, stop=True)
        br_sb = spool.tile([P, c.B], f32, tag="br", name="br")
        nc.vector.tensor_copy(br_sb[:], brp[:])

        lg = psum.tile([c.NCLS, c.B], f32, tag="ps", name="lg")
        for ot in range(c.CT):
            t1 = spool.tile([P, c.B], f32, tag="ct1", name="ct1")
            nc.vector.tensor_sub(t1[:], h_sb[:, ot, :], bm_sb[:])
            t2 = spool.tile([P, c.B], f32, tag="ct2", name="ct2")
            nc.vector.tensor_mul(t2[:], t1[:], br_sb[:])
            hn = spool.tile([P, c.B], bf16, tag="hn", name="hn")
            nc.vector.tensor_scalar(hn[:], t2[:], g_sb_c[:, ot:ot + 1],
                                    bcol_sb[:, ot:ot + 1], OP.mult, OP.add)
            w2w = wpool.tile([P, c.NCLS], bf16, tag="w2w", name="w2w")
            nc.sync.dma_start(out=w2w[:], in_=w2t[ot])
            nc.tensor.matmul(lg[:], w2w[:], hn[:],
                             start=(ot == 0), stop=(ot == c.CT - 1))
        lg_sb = spool.tile([c.NCLS, c.B], f32, tag="lg_sb", name="lg_sb")
        nc.vector.tensor_scalar(lg_sb[:], lg[:], b2_sb[:], None, OP.add)
        nc.sync.dma_start(out=out_d, in_=lg_sb[:])

    nc.compile()
    return nc


# ----------------------------------------------------------------------------
# entry point
# ----------------------------------------------------------------------------

_CACHE = {}


def _get_nc(cfg):
    if cfg not in _CACHE:
        _CACHE[cfg] = build_nc(cfg)
    return _CACHE[cfg]


def run(cfg, inputs, trace=False, **kw):
    from concourse.bass_utils import run_bass_kernel_spmd
    in_maps = host_prep(cfg, inputs)
    nc = _get_nc(cfg)
    res = run_bass_kernel_spmd(nc, in_maps, core_ids=list(range(cfg.NC)),
                               trace=trace, **kw)
    out = np.asarray(res.results[0]["logits_out"])  # [NCLS, B]
    return np.ascontiguousarray(out.T.astype(np.float32)), res


def kernel(**inputs):
    inputs = {k: np.asarray(v) for k, v in inputs.items()}
    out, _ = run(FULL_CFG, inputs)
    return out



# revision 43
# speedup vs baseline: 1.1548x; 1.1548x over previous
"""Trainium2 Bass kernel for nn_Net_89361089561102 (2-layer dense transformer,
NF4-quantized weights, cls head). Tensor-parallel over 8 NeuronCores.

Strategy:
 - Host: unpack NF4 weights -> bf16, pre-transpose to [K, M] layout, shard
   TP-style (qkv/gate_up by output dim == heads/ff, o/down by output dim),
   embedding gather, RoPE cos/sin tables, causal masks.
 - Device (per core, feature-major activations [H partitions, tokens free]):
   rmsnorm (PE ones-matmul partition reductions), qkv projections, RoPE,
   attention with transposed scores [tk, tq] (softmax denominator via PE),
   AllGather(ctx) -> o_proj (output-sharded) -> AllGather(x), gated MLP with
   AllGather(intermediate) -> down (output-sharded) -> AllGather(x).
 - Layer 2 (last layer): q/o/MLP evaluated only at the last token of each
   batch (the only positions that reach the output); k/v still full.
 - Final rmsnorm + cls head (Linear-ReLU-LayerNorm-Linear) computed
   redundantly on every core for the 2 last tokens.
"""

import math
from contextlib import ExitStack
from dataclasses import dataclass

import numpy as np
import ml_dtypes

BF16 = ml_dtypes.bfloat16
EPS = 1e-5
BLK = 64
NF4 = np.array([
    -1.0, -0.6961928009986877, -0.5250730514526367, -0.39491748809814453,
    -0.28444138169288635, -0.18477343022823334, -0.09105003625154495, 0.0,
    0.07958029955625534, 0.16093020141124725, 0.24611230194568634,
    0.33791524171829224, 0.44070982933044434, 0.5626170039176941,
    0.7229568362236023, 1.0], dtype=np.float32)


@dataclass(frozen=True)
class Cfg:
    H: int
    NH: int
    HD: int
    FF: int
    B: int
    S: int
    L: int
    NC: int
    CLS: int = 768
    NCLS: int = 2
    P: int = 128

    @property
    def T(self):
        return self.B * self.S

    @property
    def KT(self):
        return self.H // self.P

    @property
    def KTF(self):
        return self.FF // self.P

    @property
    def HPC(self):  # heads per core
        return self.NH // self.NC

    @property
    def DR(self):  # q/k/v rows per core
        return self.HPC * self.HD

    @property
    def OR(self):  # o/down output rows per core
        return self.H // self.NC

    @property
    def OT(self):
        return self.OR // self.P

    @property
    def FPC(self):  # ff rows per core
        return self.FF // self.NC

    @property
    def FT(self):
        return self.FPC // self.P

    @property
    def SP(self):  # seq tiles per batch
        return self.S // self.P

    @property
    def TP_(self):  # token tiles total
        return self.T // self.P

    @property
    def CT(self):
        return self.CLS // self.P

    def check(self):
        assert self.H % self.P == 0 and self.FF % self.P == 0
        assert self.S % self.P == 0 and self.S <= 512
        assert self.NH % self.NC == 0 and self.H % self.NC == 0
        assert self.FF % self.NC == 0
        assert self.OR % self.P == 0 and self.FPC % self.P == 0
        assert self.HD <= self.P and self.HD % 2 == 0
        assert self.HPC * self.B <= 8  # q/k psum groups
        assert self.TP_ <= 8           # v psum groups
        assert self.OT * self.B <= 8   # o/down psum groups
        assert self.CLS % self.P == 0


FULL_CFG = Cfg(H=3072, NH=32, HD=96, FF=8192, B=2, S=512, L=2, NC=8)


# ----------------------------------------------------------------------------
# host-side prep
# ----------------------------------------------------------------------------

def dequant_np(packed, absmax, out_f, in_f):
    shifts = (np.arange(8, dtype=np.int32) * 4)
    codes = ((packed[:, None] >> shifts) & 0xF).reshape(-1)
    w = (NF4[codes].reshape(-1, BLK) * absmax[:, None].astype(np.float32))
    return w.reshape(out_f, in_f)


def _wt3(w_t, P):
    """[K, M] fp32 -> [K//P, P, M] bf16 contiguous."""
    K, M = w_t.shape
    return np.ascontiguousarray(w_t.reshape(K // P, P, M).astype(BF16))


def host_prep(cfg: Cfg, inputs):
    """Full inputs -> list of per-core input maps."""
    c = cfg
    P = c.P
    x = inputs["embed"][inputs["input_ids"]]          # [B, S, H] fp32
    x0f = np.ascontiguousarray(x.reshape(c.T, c.H).T.astype(np.float32))
    x0 = np.ascontiguousarray(x0f.astype(BF16))        # [H, T] bf16

    # rope tables
    inv = 1.0 / (10000.0 ** (np.arange(0, c.HD, 2, dtype=np.float32) / c.HD))
    f = np.outer(np.arange(c.S, dtype=np.float32), inv)
    emb = np.concatenate([f, f], -1)                   # [S, HD]
    sgn = np.concatenate([-np.ones(c.HD // 2, np.float32),
                          np.ones(c.HD // 2, np.float32)])
    cosT = np.tile(np.cos(emb).T, (1, c.B))            # [HD, T]
    sinT = np.tile(np.sin(emb).T * sgn[:, None], (1, c.B))
    last = np.array([b * c.S + c.S - 1 for b in range(c.B)])
    cosT2 = np.ascontiguousarray(cosT[:, last].astype(np.float32))
    sinT2 = np.ascontiguousarray(sinT[:, last].astype(np.float32))
    cosT = np.ascontiguousarray(cosT.astype(BF16))
    sinT = np.ascontiguousarray(sinT.astype(BF16))

    am = (inputs["attention_mask"] != 0)               # [B, S]
    tk = np.arange(c.S)
    m1 = np.zeros((c.B, c.SP, P, c.S), np.float32)
    for b in range(c.B):
        for t in range(c.SP):
            rows = tk[t * P:(t + 1) * P]
            m1[b, t] = ((rows[:, None] <= tk[None, :]) & am[b, rows][:, None])
    m1 = m1.astype(BF16)
    am2 = np.zeros((c.B, P, c.SP), np.float32)
    for b in range(c.B):
        am2[b] = am[b].reshape(c.SP, P).T
    am2 = am2.astype(BF16)

    # layernorm weights [5, P, KT]
    lnw = np.zeros((2 * c.L + 1, P, c.KT), np.float32)
    for l in range(c.L):
        lnw[2 * l] = inputs["ln1_w"][l].reshape(c.KT, P).T
        lnw[2 * l + 1] = inputs["ln2_w"][l].reshape(c.KT, P).T
    lnw[2 * c.L] = inputs["final_ln_w"].reshape(c.KT, P).T

    # cls head
    w1t = _wt3(inputs["w1"].astype(np.float32).T, P)       # [KT, P, CLS]
    b1c = np.ascontiguousarray(
        inputs["b1"].reshape(c.CT, P).T.astype(np.float32))
    gcol = np.ascontiguousarray(
        inputs["ln_g"].reshape(c.CT, P).T.astype(np.float32))
    bcol = np.ascontiguousarray(
        inputs["ln_b"].reshape(c.CT, P).T.astype(np.float32))
    w2t = np.ascontiguousarray(
        inputs["w2"].astype(np.float32).T.reshape(c.CT, P, c.NCLS).astype(BF16))
    b2c = np.ascontiguousarray(
        inputs["b2"].reshape(c.NCLS, 1).astype(np.float32))
    eye2 = np.eye(c.B, dtype=np.float32)
    eye2b = eye2.astype(BF16)

    shared = dict(x0=x0, cosT=cosT, sinT=sinT, cosT2=cosT2, sinT2=sinT2,
                  m1=m1, am2=am2, lnw=lnw, w1t=w1t, b1c=b1c, gcol=gcol,
                  bcol=bcol, w2t=w2t, b2c=b2c, eye2=eye2, eye2b=eye2b)

    # per-layer dequantized weights (full) then shard
    per_layer = []
    for l in range(c.L):
        wqkv = dequant_np(inputs["qkv_packed"][l], inputs["qkv_absmax"][l],
                          3 * c.H, c.H)
        wo = dequant_np(inputs["o_packed"][l], inputs["o_absmax"][l],
                        c.H, c.H)
        wgu = dequant_np(inputs["gu_packed"][l], inputs["gu_absmax"][l],
                         2 * c.FF, c.H)
        wd = dequant_np(inputs["down_packed"][l], inputs["down_absmax"][l],
                        c.H, c.FF)
        per_layer.append((wqkv, wo, wgu, wd))

    in_maps = []
    for core in range(c.NC):
        m = dict(shared)
        m["x0r"] = np.ascontiguousarray(
            x0f[core * c.OR:(core + 1) * c.OR, :])
        for l in range(c.L):
            wqkv, wo, wgu, wd = per_layer[l]
            d0 = core * c.DR
            m[f"wq{l}"] = _wt3(wqkv[d0:d0 + c.DR, :].T, P)
            m[f"wk{l}"] = _wt3(wqkv[c.H + d0:c.H + d0 + c.DR, :].T, P)
            m[f"wv{l}"] = _wt3(wqkv[2 * c.H + d0:2 * c.H + d0 + c.DR, :].T, P)
            o0 = core * c.OR
            m[f"wo{l}"] = _wt3(wo[o0:o0 + c.OR, :].T, P)
            g0 = core * c.FPC
            m[f"wg{l}"] = _wt3(wgu[g0:g0 + c.FPC, :].T, P)
            m[f"wu{l}"] = _wt3(wgu[c.FF + g0:c.FF + g0 + c.FPC, :].T, P)
            m[f"wd{l}"] = _wt3(wd[o0:o0 + c.OR, :].T, P)
        # last-layer slim path: o/down sharded by INPUT dim (this core's
        # ctx heads / ff rows), full output rows -> partial sums + AllReduce
        wqkv, wo, wgu, wd = per_layer[c.L - 1]
        d0 = core * c.DR
        wos = np.zeros((c.HPC, P, c.H), np.float32)
        for h in range(c.HPC):
            wos[h, 0:c.HD, :] = wo[:, d0 + h * c.HD:d0 + (h + 1) * c.HD].T
        m["wos"] = np.ascontiguousarray(wos.astype(BF16))
        f0 = core * c.FPC
        m["wds"] = np.ascontiguousarray(
            wd[:, f0:f0 + c.FPC].T.reshape(c.FT, P, c.H).astype(BF16))
        in_maps.append(m)
    return in_maps


# ----------------------------------------------------------------------------
# device kernel
# ----------------------------------------------------------------------------

def build_nc(cfg: Cfg):
    import concourse.bass as bass
    import concourse.mybir as mybir
    import concourse.tile as tile
    from concourse import bacc

    c = cfg
    c.check()
    P = c.P
    f32 = mybir.dt.float32
    bf16 = mybir.dt.bfloat16
    AF = mybir.ActivationFunctionType
    OP = mybir.AluOpType

    nc = bacc.Bacc("TRN2", target_bir_lowering=False, debug=False,
                   enable_asserts=False, num_devices=c.NC)
    RG = [list(range(c.NC))]
    SHARED = "Shared" if c.NC > 4 else "Local"

    def din(name, shape, dt):
        return nc.dram_tensor(name, list(shape), dt, kind="ExternalInput").ap()

    x0 = din("x0", [c.H, c.T], bf16)
    x0r = din("x0r", [c.OR, c.T], f32)
    cosT = din("cosT", [c.HD, c.T], bf16)
    sinT = din("sinT", [c.HD, c.T], bf16)
    cosT2 = din("cosT2", [c.HD, c.B], f32)
    sinT2 = din("sinT2", [c.HD, c.B], f32)
    m1 = din("m1", [c.B, c.SP, P, c.S], bf16)
    am2 = din("am2", [c.B, P, c.SP], bf16)
    lnw_d = din("lnw", [2 * c.L + 1, P, c.KT], f32)
    w1t = din("w1t", [c.KT, P, c.CLS], bf16)
    b1c = din("b1c", [P, c.CT], f32)
    gcol = din("gcol", [P, c.CT], f32)
    bcol = din("bcol", [P, c.CT], f32)
    w2t = din("w2t", [c.CT, P, c.NCLS], bf16)
    b2c = din("b2c", [c.NCLS, 1], f32)
    eye2_d = din("eye2", [c.B, c.B], f32)
    eye2b_d = din("eye2b", [c.B, c.B], bf16)
    wos_d = din("wos", [c.HPC, P, c.H], bf16)
    wds_d = din("wds", [c.FT, P, c.H], bf16)
    wq = [din(f"wq{l}", [c.KT, P, c.DR], bf16) for l in range(c.L)]
    wk = [din(f"wk{l}", [c.KT, P, c.DR], bf16) for l in range(c.L)]
    wv = [din(f"wv{l}", [c.KT, P, c.DR], bf16) for l in range(c.L)]
    wo = [din(f"wo{l}", [c.KT, P, c.OR], bf16) for l in range(c.L)]
    wg = [din(f"wg{l}", [c.KT, P, c.FPC], bf16) for l in range(c.L)]
    wu = [din(f"wu{l}", [c.KT, P, c.FPC], bf16) for l in range(c.L)]
    wd = [din(f"wd{l}", [c.KTF, P, c.OR], bf16) for l in range(c.L)]
    out_d = nc.dram_tensor("logits_out", [c.NCLS, c.B], f32,
                           kind="ExternalOutput").ap()

    isqrt_hd = 1.0 / math.sqrt(c.HD)

    def lastcols(ap2d):
        """[P, T] AP -> [P, B] AP selecting the last token of each batch."""
        return ap2d.rearrange("p (b s) -> p b s", s=c.S)[:, :, c.S - 1]

    with tile.TileContext(nc) as tc, ExitStack() as ctx:
        const = ctx.enter_context(tc.tile_pool(name="const", bufs=1))
        persist = ctx.enter_context(tc.tile_pool(name="persist", bufs=1))
        wpool = ctx.enter_context(tc.tile_pool(name="wpool", bufs=3))
        xpool = ctx.enter_context(tc.tile_pool(name="xpool", bufs=3))
        spool = ctx.enter_context(tc.tile_pool(name="spool", bufs=2))
        ppool = ctx.enter_context(tc.tile_pool(name="ppool", bufs=3))
        rpool = ctx.enter_context(tc.tile_pool(name="rpool", bufs=1))
        psum = ctx.enter_context(tc.tile_pool(name="psum", bufs=8,
                                              space="PSUM"))
        dram = ctx.enter_context(tc.tile_pool(name="dram", bufs=1,
                                              space="DRAM"))

        # ---- constants in SBUF ----
        ones_c32 = const.tile([P, 1], f32, tag="ones_c32")
        nc.vector.memset(ones_c32[:], 1.0)
        ones_cbf = const.tile([P, 1], bf16, tag="ones_cbf")
        nc.vector.memset(ones_cbf[:], 1.0)
        ones_r32 = const.tile([1, P], f32, tag="ones_r32")
        nc.vector.memset(ones_r32[:], 1.0)
        eps_col = const.tile([P, 1], f32, tag="eps_col")
        nc.vector.memset(eps_col[:], EPS)
        cos_sb = const.tile([c.HD, c.T], bf16, tag="cos_sb")
        nc.sync.dma_start(out=cos_sb[:], in_=cosT)
        sin_sb = const.tile([c.HD, c.T], bf16, tag="sin_sb")
        nc.sync.dma_start(out=sin_sb[:], in_=sinT)
        cos2_sb = const.tile([c.HD, c.B], f32, tag="cos2_sb")
        nc.sync.dma_start(out=cos2_sb[:], in_=cosT2)
        sin2_sb = const.tile([c.HD, c.B], f32, tag="sin2_sb")
        nc.sync.dma_start(out=sin2_sb[:], in_=sinT2)
        mask_sb = const.tile([P, c.B * c.SP, c.S], bf16, tag="mask_sb")
        for b in range(c.B):
            for t in range(c.SP):
                nc.scalar.dma_start(out=mask_sb[:, b * c.SP + t, :],
                                    in_=m1[b, t])
        am2_sb = const.tile([P, c.B, c.SP], bf16, tag="am2_sb")
        for b in range(c.B):
            nc.sync.dma_start(out=am2_sb[:, b, :], in_=am2[b])
        lnw_sb = const.tile([P, 2 * c.L + 1, c.KT], f32, tag="lnw_sb")
        for n in range(2 * c.L + 1):
            nc.sync.dma_start(out=lnw_sb[:, n, :], in_=lnw_d[n])
        b1_sb = const.tile([P, c.CT], f32, tag="b1_sb")
        nc.sync.dma_start(out=b1_sb[:], in_=b1c)
        g_sb_c = const.tile([P, c.CT], f32, tag="g_sb_c")
        nc.sync.dma_start(out=g_sb_c[:], in_=gcol)
        bcol_sb = const.tile([P, c.CT], f32, tag="bcol_sb")
        nc.sync.dma_start(out=bcol_sb[:], in_=bcol)
        b2_sb = const.tile([c.NCLS, 1], f32, tag="b2_sb")
        nc.sync.dma_start(out=b2_sb[:], in_=b2c)
        eye2_sb = const.tile([c.B, c.B], f32, tag="eye2_sb")
        nc.sync.dma_start(out=eye2_sb[:], in_=eye2_d)
        eye2b_sb = const.tile([c.B, c.B], bf16, tag="eye2b_sb")
        nc.sync.dma_start(out=eye2b_sb[:], in_=eye2b_d)

        # ---- collective warm-up: absorb channel-establish cost under
        # the first compute phase (first real AG otherwise pays ~200us) ----
        wu_sb = const.tile([P, 512], f32, tag="wu_sb")
        nc.vector.memset(wu_sb[:], 0.0)
        wu_in = dram.tile([P, 512], f32, tag="wu_in", name="wu_in")
        wu_out = dram.tile([P * c.NC, 512], f32, addr_space=SHARED,
                           tag="wu_out", name="wu_out")
        nc.sync.dma_start(out=wu_in[:], in_=wu_sb[:])
        nc.gpsimd.collective_compute(
            "AllGather", OP.bypass, replica_groups=RG,
            ins=[wu_in[:]], outs=[wu_out[:]])

        # ---- persistent activation state ----
        xn = persist.tile([P, c.KT, c.T], bf16, tag="xn")       # normalized x (bf16)
        xrows = persist.tile([P, c.OT, c.T], f32, tag="xrows")     # this core's rows of x
        for ot in range(c.OT):
            nc.sync.dma_start(out=xrows[:, ot, :],
                              in_=x0r[ot * P:(ot + 1) * P, :])

        # ---------- helpers ----------
        def emit_norm(src_ap, lnidx, dst, ncols, chunks):
            """rmsnorm of src [H, ncols] (bf16) -> dst [P, KT, ncols] (bf16).
            Chunk-wise so working tiles stay <= 512 cols wide."""
            ss = [psum.tile([1, cw], f32, tag="ps", name=f"ssps{lnidx}_{ci}")
                  for ci, (c0, cw) in enumerate(chunks)]
            for kt in range(c.KT):
                for ci, (c0, cw) in enumerate(chunks):
                    xf = xpool.tile([P, cw], bf16, tag="xf", name="xf",
                                    bufs=2)
                    nc.sync.dma_start(
                        out=xf[:], in_=src_ap[kt * P:(kt + 1) * P,
                                              c0:c0 + cw])
                    nc.vector.tensor_copy(dst[:, kt, c0:c0 + cw], xf[:])
                    sq = xpool.tile([P, cw], bf16, tag="sq", name="sq",
                                    bufs=2)
                    nc.vector.tensor_mul(sq[:], xf[:], xf[:])
                    nc.tensor.matmul(ss[ci][:], ones_cbf[:], sq[:],
                                     start=(kt == 0), stop=(kt == c.KT - 1))
            for ci, (c0, cw) in enumerate(chunks):
                lt = spool.tile([1, cw], f32, tag="lt", name="lt")
                nc.scalar.activation(lt[:], ss[ci][:], AF.Ln,
                                     bias=eps_col[0:1, :], scale=1.0 / c.H)
                rt = spool.tile([1, cw], f32, tag="rt", name="rt")
                nc.scalar.activation(rt[:], lt[:], AF.Exp, scale=-0.5)
                bb = psum.tile([P, cw], f32, tag="ps", name="bbps")
                nc.tensor.matmul(bb[:], ones_r32[:], rt[:],
                                 start=True, stop=True)
                bc = spool.tile([P, cw], f32, tag="bc", name="bc", bufs=2)
                nc.scalar.copy(bc[:], bb[:])
                for kt in range(c.KT):
                    nc.vector.scalar_tensor_tensor(
                        dst[:, kt, c0:c0 + cw], dst[:, kt, c0:c0 + cw],
                        lnw_sb[:, lnidx, kt:kt + 1], bc[:],
                        OP.mult, OP.mult)

        def emit_rstd_bcast(ss_aps, lnidx, ncols, chunks):
            """ss_aps: per-chunk [1, cw] APs of full-H sum-of-squares.
            Returns bc_sb [P, ncols] f32 with rsqrt(mean+eps) per token."""
            bc = spool.tile([P, ncols], f32, tag="bc", name="bc", bufs=2)
            for ci, (c0, cw) in enumerate(chunks):
                lt = spool.tile([1, cw], f32, tag="lt", name="lt")
                nc.scalar.activation(lt[:], ss_aps[ci], AF.Ln,
                                     bias=eps_col[0:1, :], scale=1.0 / c.H)
                rt = spool.tile([1, cw], f32, tag="rt", name="rt")
                nc.scalar.activation(rt[:], lt[:], AF.Exp, scale=-0.5)
                bb = psum.tile([P, cw], f32, tag="ps", name="bbps")
                nc.tensor.matmul(bb[:], ones_r32[:], rt[:],
                                 start=True, stop=True)
                nc.scalar.copy(bc[:, c0:c0 + cw], bb[:])
            return bc

        def emit_sumsq_ar(x0_, cw, tag):
            """Partial sum-of-squares of this core's fp32 x rows over token
            columns [x0_, x0_+cw), then a tiny AllReduce (issued before the
            x AllGather so the norm scale is ready when x streams back)."""
            ssq = psum.tile([1, cw], f32, tag="ps", name=f"ssA{tag}")
            for ot in range(c.OT):
                sqr = xpool.tile([P, cw], bf16, tag="sqr", name="sqr",
                                 bufs=2)
                nc.vector.tensor_mul(sqr[:], xrows[:, ot, x0_:x0_ + cw],
                                     xrows[:, ot, x0_:x0_ + cw])
                nc.tensor.matmul(ssq[:], ones_cbf[:], sqr[:],
                                 start=(ot == 0), stop=(ot == c.OT - 1))
            srow = spool.tile([1, cw], f32, tag="srow", name="srow", bufs=2)
            nc.scalar.copy(srow[:], ssq[:])
            ssb = dram.tile([1, cw], f32, tag=f"ssb{tag}",
                            name=f"ssb{tag}")
            ssg = dram.tile([1, cw], f32, addr_space=SHARED,
                            tag=f"ssg{tag}", name=f"ssg{tag}")
            nc.sync.dma_start(out=ssb[:], in_=srow[:])
            nc.gpsimd.collective_compute(
                "AllReduce", OP.add, replica_groups=RG,
                ins=[ssb[:]], outs=[ssg[:]])
            return ssg

        def emit_norm_post(ssg, src_ap, lnidx, dst, dst_c0, ncols, chunks):
            """normalize src [H, ncols] into dst[:, kt, dst_c0:dst_c0+ncols]"""
            sst = spool.tile([1, ncols], f32, tag="sst", name="sst", bufs=2)
            nc.sync.dma_start(out=sst[:], in_=ssg[:])
            bc = emit_rstd_bcast(
                [sst[:, c0:c0 + cw] for (c0, cw) in chunks],
                lnidx, ncols, chunks)
            for kt in range(c.KT):
                xf = xpool.tile([P, ncols], bf16, tag="xf", name="xfa",
                                bufs=2)
                nc.sync.dma_start(out=xf[:],
                                  in_=src_ap[kt * P:(kt + 1) * P, :])
                nc.vector.scalar_tensor_tensor(
                    dst[:, kt, dst_c0:dst_c0 + ncols], xf[:],
                    lnw_sb[:, lnidx, kt:kt + 1],
                    bc[:], OP.mult, OP.mult)

        def emit_norm_slim(xs, lnidx, dst3):
            """rmsnorm of an SBUF [P, KT, B] tile: local sumsq."""
            sq = spool.tile([P, c.KT, c.B], f32, tag="sq_slim",
                            name="sq_slim")
            nc.vector.tensor_mul(sq[:], xs[:], xs[:])
            sp_ = psum.tile([1, c.KT * c.B], f32, tag="ps", name="spslim")
            nc.tensor.matmul(sp_[:], ones_c32[:],
                             sq[:].rearrange("p kt b -> p (kt b)"),
                             start=True, stop=True)
            ss2 = spool.tile([1, c.B], f32, tag="ss2", name="ss2")
            nc.vector.tensor_reduce(
                ss2[:], sp_[:].rearrange("o (kt b) -> o b kt", b=c.B),
                mybir.AxisListType.X, OP.add)
            bc = emit_rstd_bcast([ss2[:]], lnidx, c.B, [(0, c.B)])
            tmp = spool.tile([P, c.KT, c.B], f32, tag="tmp_slim",
                             name="tmp_slim")
            nc.vector.tensor_tensor(
                tmp[:], xs[:],
                lnw_sb[:, lnidx, :].unsqueeze(2).broadcast_to(
                    (P, c.KT, c.B)), OP.mult)
            nc.vector.tensor_tensor(
                dst3[:], tmp[:],
                bc[:].unsqueeze(1).broadcast_to((P, c.KT, c.B)), OP.mult)

        def kouter_pass(KK, wsrc, wcols, groups, rhs_fn, rhs_load=None,
                        name="kp"):
            """Generic contraction pass: loop k tiles (batched weight DMA),
            stream weights, accumulate len(groups) psum tiles.
            groups: list of (lhs_c0, lhs_cw, out_n, rhs_key)."""
            ps = [psum.tile([cw, n], f32, tag="ps", name=f"{name}{gi}")
                  for gi, (c0, cw, n, rk) in enumerate(groups)]
            G = max(1, min(4, 2048 // wcols))
            for k0 in range(0, KK, G):
                g_n = min(G, KK - k0)
                wt = wpool.tile([P, G, 2048 // G if wcols > 2048 // G else wcols],
                                bf16, tag="wt", name=f"{name}w", bufs=2)
                nc.scalar.dma_start(
                    out=wt[:, 0:g_n, 0:wcols],
                    in_=wsrc(k0, g_n).rearrange("g p m -> p g m"))
                for g in range(g_n):
                    kt = k0 + g
                    rl = rhs_load(kt) if rhs_load is not None else None
                    for gi, (c0, cw, n, rk) in enumerate(groups):
                        nc.tensor.matmul(ps[gi][:], wt[:, g, c0:c0 + cw],
                                         rhs_fn(kt, rk, rl),
                                         start=(kt == 0), stop=(kt == KK - 1))
            return ps

        def emit_rope(src_ps, qr_dst, cos_ap, sin_ap, ncols):
            """rope: qr_dst = src*cos + swap_half(src)*sin_signed."""
            h2 = c.HD // 2
            qs = rpool.tile([c.HD, ncols], bf16, tag="qs", name="qs")
            nc.vector.tensor_copy(qs[:], src_ps[:])
            rot = rpool.tile([c.HD, ncols], bf16, tag="rot", name="rot")
            nc.sync.dma_start(out=rot[0:h2, :], in_=qs[h2:c.HD, :])
            nc.sync.dma_start(out=rot[h2:c.HD, :], in_=qs[0:h2, :])
            nc.vector.tensor_mul(qs[:], qs[:], cos_ap)
            nc.vector.tensor_mul(rot[:], rot[:], sin_ap)
            nc.vector.tensor_add(qr_dst, qs[:], rot[:])

        # ================= transformer layers =================
        x_src = x0
        ln1_ssg = None
        for l in range(c.L):
            slim = (l == c.L - 1)
            ncol2 = c.B if slim else c.T
            full_chunks = [(b * c.S, c.S) for b in range(c.B)]

            # resident q/k/v weights for this layer (DMA'd early, big xfers)
            wq_sb = None
            if not slim:
                wq_sb = wpool.tile([P, c.KT, c.DR], bf16, tag="wq_sb",
                                   name=f"wq_sb{l}", bufs=1)
                nc.scalar.dma_start(out=wq_sb[:],
                                    in_=wq[l].rearrange("kt p m -> p kt m"))
            wk_sb = wpool.tile([P, c.KT, c.DR], bf16, tag="wk_sb",
                               name=f"wk_sb{l}", bufs=1)
            nc.scalar.dma_start(out=wk_sb[:],
                                in_=wk[l].rearrange("kt p m -> p kt m"))

            # ---- ln1 + qkv ----
            if ln1_ssg is None:
                emit_norm(x_src, 2 * l, xn, c.T, full_chunks)
            else:
                # per-batch: x_src is a list of per-batch [H, S] gathers
                for b in range(c.B):
                    emit_norm_post(ln1_ssg[b], x_src[b], 2 * l, xn,
                                   b * c.S, c.S, [(0, c.S)])

            q_rot = persist.tile([c.HD, c.HPC, ncol2], bf16, tag="qrot",
                                 name=f"qrot{l}")
            k_rot = persist.tile([c.HD, c.HPC, c.T], bf16, tag="krot",
                                 name=f"krot{l}")
            v_sb = persist.tile([P, c.TP_, c.DR], bf16, tag="vsb",
                                name=f"vsb{l}")

            # q pass (slim: only last token of each batch)
            if slim:
                qg = [(h * c.HD, c.HD, c.B, 0) for h in range(c.HPC)]
                qrhs = lambda kt, rk, rl: lastcols(xn[:, kt, :])
                qps = kouter_pass(c.KT, lambda k0, n: wq[l][k0:k0 + n], c.DR,
                                  qg, qrhs, name="qp")
                for gi, (c0, cw, n, rk) in enumerate(qg):
                    h = c0 // c.HD
                    emit_rope(qps[gi], q_rot[:, h, :], cos2_sb[:], sin2_sb[:],
                              c.B)
            else:
                # head-outer, K-contiguous: rope of head i overlaps matmuls
                # of head i+1
                for h in range(c.HPC):
                    for b in range(c.B):
                        qp = psum.tile([c.HD, c.S], f32, tag="ps",
                                       name="qhps")
                        for kt in range(c.KT):
                            nc.tensor.matmul(
                                qp[:], wq_sb[:, kt, h * c.HD:(h + 1) * c.HD],
                                xn[:, kt, b * c.S:(b + 1) * c.S],
                                start=(kt == 0), stop=(kt == c.KT - 1))
                        emit_rope(qp, q_rot[:, h, b * c.S:(b + 1) * c.S],
                                  cos_sb[:, b * c.S:(b + 1) * c.S],
                                  sin_sb[:, b * c.S:(b + 1) * c.S], c.S)

            # k pass (always full), head-outer
            for h in range(c.HPC):
                for b in range(c.B):
                    kp = psum.tile([c.HD, c.S], f32, tag="ps",
                                   name="khps")
                    for kt in range(c.KT):
                        nc.tensor.matmul(
                            kp[:], wk_sb[:, kt, h * c.HD:(h + 1) * c.HD],
                            xn[:, kt, b * c.S:(b + 1) * c.S],
                            start=(kt == 0), stop=(kt == c.KT - 1))
                    emit_rope(kp, k_rot[:, h, b * c.S:(b + 1) * c.S],
                              cos_sb[:, b * c.S:(b + 1) * c.S],
                              sin_sb[:, b * c.S:(b + 1) * c.S], c.S)

            # v pass (token-major): psum groups per token tile
            vps = [psum.tile([P, c.DR], f32, tag="ps", name=f"vp{tt}")
                   for tt in range(c.TP_)]
            for kt in range(c.KT):
                wt = wpool.tile([P, c.DR], bf16, tag="wt", name="vw", bufs=2)
                nc.scalar.dma_start(out=wt[:], in_=wv[l][kt])
                for tt in range(c.TP_):
                    nc.tensor.matmul(vps[tt][:],
                                     xn[:, kt, tt * P:(tt + 1) * P], wt[:],
                                     start=(kt == 0), stop=(kt == c.KT - 1))
            for tt in range(c.TP_):
                nc.scalar.copy(v_sb[:, tt, :], vps[tt][:])

            # ---- attention ----
            if slim:
                # ctx stays local in SBUF (o is input-sharded; partitions
                # HD..P zero so padded o rows contribute nothing)
                ctx_sb = persist.tile([P, c.HPC, c.B], bf16, tag="ctx_sb",
                                      name="ctx_sb")
                nc.vector.memset(ctx_sb[:], 0.0)
                ctxbs, ctxgs = [], []
            else:
                ctxbs = [dram.tile([c.DR, c.S], bf16, tag=f"ctxb{l}_{b}",
                                   name=f"ctxb{l}_{b}") for b in range(c.B)]
                ctxgs = [dram.tile([c.H, c.S], bf16, addr_space=SHARED,
                                   tag=f"ctxg{l}_{b}", name=f"ctxg{l}_{b}")
                         for b in range(c.B)]
            def att_phase1(b, h):
                """scores -> exp -> mask; returns masked prob tiles."""
                pts = []
                for t in range(c.SP):
                    n0 = t * P  # causal: tile t only sees tq >= t*P
                    sps = psum.tile([P, c.S], f32, tag="ps", name="sps")
                    nc.tensor.matmul(
                        sps[:, n0:],
                        k_rot[:, h, b * c.S + t * P:b * c.S + (t + 1) * P],
                        q_rot[:, h, b * c.S + n0:(b + 1) * c.S],
                        start=True, stop=True)
                    pt = ppool.tile([P, c.S], bf16, tag="pt", name="pt",
                                    bufs=5)
                    nc.scalar.activation(pt[:, n0:], sps[:, n0:],
                                         AF.Exp, scale=isqrt_hd)
                    nc.vector.tensor_mul(
                        pt[:, n0:], pt[:, n0:],
                        mask_sb[:, b * c.SP + t, n0:])
                    pts.append(pt)
                return pts

            def att_phase2(b, h, pts):
                """denominator + AV + normalize + store ctx."""
                den = psum.tile([1, c.S], f32, tag="ps", name="den")
                cps = psum.tile([c.HD, c.S], f32, tag="ps", name="cps")
                for t, pt in enumerate(pts):
                    n0 = t * P
                    nc.tensor.matmul(den[:, n0:], ones_cbf[:], pt[:, n0:],
                                     start=(t == 0), stop=(t == c.SP - 1))
                    nc.tensor.matmul(
                        cps[:, n0:],
                        v_sb[:, b * c.SP + t, h * c.HD:(h + 1) * c.HD],
                        pt[:, n0:],
                        start=(t == 0), stop=(t == c.SP - 1))
                # 1/den via Ln+Exp on Scalar (DVE reciprocal on a
                # 1-partition tile is ~3.4us; this is ~1.3us)
                lt = spool.tile([1, c.S], f32, tag="dr", name="dln")
                nc.scalar.activation(lt[:], den[:], AF.Ln)
                dr = spool.tile([1, c.S], f32, tag="dr", name="dr")
                nc.scalar.activation(dr[:], lt[:], AF.Exp, scale=-1.0)
                bb = psum.tile([c.HD, c.S], f32, tag="ps", name="bb")
                nc.tensor.matmul(bb[:], ones_r32[:, 0:c.HD], dr[:],
                                 start=True, stop=True)
                bsb = spool.tile([c.HD, c.S], f32, tag="csb",
                                 name="bsb", bufs=2)
                nc.scalar.copy(bsb[:], bb[:])
                csb = spool.tile([c.HD, c.S], bf16, tag="csb",
                                 name="csb", bufs=2)
                nc.vector.tensor_mul(csb[:], cps[:], bsb[:])
                nc.sync.dma_start(
                    out=ctxbs[b][h * c.HD:(h + 1) * c.HD, :],
                    in_=csb[:])
                if h == c.HPC - 1:
                    nc.gpsimd.collective_compute(
                        "AllGather", OP.bypass, replica_groups=RG,
                        ins=[ctxbs[b][:]], outs=[ctxgs[b][:]])

            if not slim:
                # 2-stage software pipeline: head i+1's scores/exp/mask
                # overlap head i's den/AV/normalize on the other engines
                pend = None
                for bh in [(b, h) for b in range(c.B)
                           for h in range(c.HPC)]:
                    cur = (bh, att_phase1(*bh))
                    if pend is not None:
                        att_phase2(pend[0][0], pend[0][1], pend[1])
                    pend = cur
                att_phase2(pend[0][0], pend[0][1], pend[1])

            for b in range(c.B if slim else 0):
                for h in range(c.HPC):
                        sps = psum.tile([P, c.SP], f32, tag="ps", name="sps2")
                        for t in range(c.SP):
                            nc.tensor.matmul(
                                sps[:, t:t + 1],
                                k_rot[:, h, b * c.S + t * P:
                                      b * c.S + (t + 1) * P],
                                q_rot[:, h, b:b + 1],
                                start=True, stop=True)
                        pt = ppool.tile([P, c.SP], bf16, tag="pt2",
                                        name="pt2")
                        nc.scalar.activation(pt[:], sps[:], AF.Exp,
                                             scale=isqrt_hd)
                        nc.vector.tensor_mul(pt[:], pt[:], am2_sb[:, b, :])
                        dps = psum.tile([1, c.SP], f32, tag="ps", name="dps")
                        nc.tensor.matmul(dps[:], ones_cbf[:], pt[:],
                                         start=True, stop=True)
                        d1 = spool.tile([1, 1], f32, tag="d1", name="d1")
                        nc.vector.tensor_reduce(d1[:], dps[:],
                                                mybir.AxisListType.X, OP.add)
                        r1 = spool.tile([1, 1], f32, tag="r1", name="r1")
                        nc.vector.reciprocal(r1[:], d1[:])
                        cps = psum.tile([c.HD, 1], f32, tag="ps", name="cps2")
                        for t in range(c.SP):
                            nc.tensor.matmul(
                                cps[:],
                                v_sb[:, b * c.SP + t,
                                     h * c.HD:(h + 1) * c.HD],
                                pt[:, t:t + 1],
                                start=(t == 0), stop=(t == c.SP - 1))
                        bb = psum.tile([c.HD, 1], f32, tag="ps", name="bb2")
                        nc.tensor.matmul(bb[:], ones_r32[:, 0:c.HD], r1[:],
                                         start=True, stop=True)
                        bsb = spool.tile([c.HD, 1], f32, tag="bsb2",
                                         name="bsb2")
                        nc.vector.tensor_copy(bsb[:], bb[:])
                        nc.vector.tensor_mul(
                            ctx_sb[0:c.HD, h, b:b + 1], cps[:], bsb[:])

            if slim:
                # 512-col chunks of the full H / FPC outputs
                hch = [(i * 512, min(512, c.H - i * 512))
                       for i in range((c.H + 511) // 512)]
                fch = [(i * 512, min(512, c.FPC - i * 512))
                       for i in range((c.FPC + 511) // 512)]
                HC = len(hch)

                def stream_w(ap2d, pcols, nm):
                    t = wpool.tile([P, pcols], bf16, tag="wt", name=nm,
                                   bufs=2)
                    nc.sync.dma_start(out=t[:], in_=ap2d)
                    return t

                def ar_round(ps_list, tag2):
                    """evacuate [B,512] psums -> DRAM, AllReduce, return
                    the gathered [B, H] f32 DRAM AP."""
                    arb = dram.tile([c.B, c.H], f32, tag=f"arb{tag2}",
                                    name=f"arb{tag2}")
                    arg_ = dram.tile([c.B, c.H], f32, addr_space=SHARED,
                                     tag=f"arg{tag2}", name=f"arg{tag2}")
                    for ci, (c0_, cw_) in enumerate(hch):
                        ev = spool.tile([c.B, cw_], f32, tag="sg1",
                                        name="ev", bufs=2)
                        nc.vector.tensor_copy(ev[:], ps_list[ci][:])
                        nc.sync.dma_start(
                            out=arb[:, c0_:c0_ + cw_], in_=ev[:])
                    nc.gpsimd.collective_compute(
                        "AllReduce", OP.add, replica_groups=RG,
                        ins=[arb[:]], outs=[arg_[:]])
                    return arg_

                def add_transposed(arg_, dst, residual_ap=None):
                    """dst[:, kt, :] (+)= transpose(arg_[:, ktP:(kt+1)P])"""
                    for kt in range(c.KT):
                        xc = spool.tile([c.B, P], f32, tag="xc", name="xc",
                                        bufs=2)
                        nc.sync.dma_start(
                            out=xc[:], in_=arg_[:, kt * P:(kt + 1) * P])
                        tp = psum.tile([P, c.B], f32, tag="ps", name="tp")
                        nc.tensor.transpose(tp[:], xc[:], eye2_sb[:])
                        if residual_ap is None:
                            nc.vector.tensor_add(dst[:, kt, :],
                                                 dst[:, kt, :], tp[:])
                        else:
                            nc.vector.tensor_add(dst[:, kt, :],
                                                 residual_ap[:, kt, :],
                                                 tp[:])

                # ---- o partial (input-sharded over this core's heads),
                # flipped: ctx [P,B] stationary, Wo^T chunks moving.
                # Weights stream in large chunks through the (now dead)
                # resident q/k weight buffers, alternating for overlap.
                def big_wbuf(i, shape, nm):
                    tag2 = "wq_sb" if i % 2 == 0 else "wk_sb"
                    return wpool.tile(shape, bf16, tag=tag2, name=nm,
                                      bufs=1)

                po = [psum.tile([c.B, cw_], f32, tag="ps",
                                name=f"po{ci}")
                      for ci, (c0_, cw_) in enumerate(hch)]
                OB = 2  # o-chunk pairs per DMA
                for ci0 in range(0, len(hch), OB):
                    cis = range(ci0, min(ci0 + OB, len(hch)))
                    c0b = hch[cis[0]][0]
                    cwb = sum(hch[ci][1] for ci in cis)
                    wt_ = big_wbuf(ci0 // OB, [P, c.HPC, cwb], f"wos{ci0}")
                    nc.scalar.dma_start(
                        out=wt_[:],
                        in_=wos_d[:, :, c0b:c0b + cwb].rearrange(
                            "h p m -> p h m"))
                    for ci in cis:
                        off = hch[ci][0] - c0b
                        cw_ = hch[ci][1]
                        for h in range(c.HPC):
                            nc.tensor.matmul(
                                po[ci][:], ctx_sb[:, h, :],
                                wt_[:, h, off:off + cw_],
                                start=(h == 0), stop=(h == c.HPC - 1))
                oarg = ar_round(po, "o")

                # residual base: last token column of each batch
                xlast = spool.tile([P, c.KT, c.B], bf16, tag="xlast",
                                   name="xlast")
                for b in range(c.B):
                    nc.sync.dma_start(
                        out=xlast[:, :, b:b + 1],
                        in_=x_src[b].rearrange(
                            "(kt p) s -> p kt s", p=P)[:, :, c.S - 1:c.S])
                xslim = persist.tile([P, c.KT, c.B], f32, tag="xslim",
                                     name="xslim")
                add_transposed(oarg, xslim, residual_ap=xlast)

                # ---- ln2 + gated MLP (flipped, output-sharded gu) ----
                xn2 = persist.tile([P, c.KT, c.B], bf16, tag="xn2",
                                   name="xn2")
                emit_norm_slim(xslim, 2 * l + 1, xn2)
                gt = [psum.tile([c.B, cw_], f32, tag="ps",
                                name=f"gt{i}")
                      for i, (c0_, cw_) in enumerate(fch)]
                ut = [psum.tile([c.B, cw_], f32, tag="ps",
                                name=f"ut{i}")
                      for i, (c0_, cw_) in enumerate(fch)]
                GB = max(1, min(4, c.KT))  # k-tiles per weight chunk
                for blk, k0 in enumerate(range(0, c.KT, GB)):
                    kn = min(GB, c.KT - k0)
                    wgu_t = big_wbuf(blk, [P, GB, 2 * c.FPC], f"wgu{blk}")
                    nc.scalar.dma_start(
                        out=wgu_t[:, 0:kn, 0:c.FPC],
                        in_=wg[l][k0:k0 + kn].rearrange("g p m -> p g m"))
                    nc.scalar.dma_start(
                        out=wgu_t[:, 0:kn, c.FPC:2 * c.FPC],
                        in_=wu[l][k0:k0 + kn].rearrange("g p m -> p g m"))
                    for g in range(kn):
                        kt = k0 + g
                        for i, (c0_, cw_) in enumerate(fch):
                            nc.tensor.matmul(gt[i][:], xn2[:, kt, :],
                                             wgu_t[:, g, c0_:c0_ + cw_],
                                             start=(kt == 0),
                                             stop=(kt == c.KT - 1))
                            nc.tensor.matmul(
                                ut[i][:], xn2[:, kt, :],
                                wgu_t[:, g, c.FPC + c0_:c.FPC + c0_ + cw_],
                                start=(kt == 0),
                                stop=(kt == c.KT - 1))
                # silu(g)*u -> transpose chunks into [P, FT, B]
                int_c = persist.tile([P, c.FT, c.B], bf16, tag="int_c",
                                     name="int_c")
                for i, (c0_, cw_) in enumerate(fch):
                    sg = spool.tile([c.B, cw_], f32, tag="sg1", name="sg",
                                    bufs=2)
                    nc.scalar.activation(sg[:], gt[i][:], AF.Sigmoid)
                    nc.vector.tensor_mul(sg[:], gt[i][:], sg[:])
                    itc = spool.tile([c.B, cw_], bf16, tag="sg3", name="itc",
                                     bufs=2)
                    nc.vector.tensor_mul(itc[:], ut[i][:], sg[:])
                    for j in range(cw_ // P):
                        tp = psum.tile([P, c.B], bf16, tag="ps", name="tpi")
                        nc.tensor.transpose(tp[:], itc[:, j * P:(j + 1) * P],
                                            eye2b_sb[:])
                        nc.vector.tensor_copy(
                            int_c[:, (c0_ // P) + j, :], tp[:])

                # ---- down partial (input-sharded over this core's ff
                # rows) + AllReduce + residual ----
                pd = [psum.tile([c.B, cw_], f32, tag="ps",
                                name=f"pd{ci}")
                      for ci, (c0_, cw_) in enumerate(hch)]
                DB = max(1, min(2, c.FT))  # ff-tiles per weight chunk
                for blk, k0 in enumerate(range(0, c.FT, DB)):
                    kn = min(DB, c.FT - k0)
                    wd_t = big_wbuf(blk, [P, DB, c.H], f"wds{blk}")
                    nc.scalar.dma_start(
                        out=wd_t[:, 0:kn, :],
                        in_=wds_d[k0:k0 + kn].rearrange("g p m -> p g m"))
                    for g in range(kn):
                        kt = k0 + g
                        for ci, (c0_, cw_) in enumerate(hch):
                            nc.tensor.matmul(pd[ci][:], int_c[:, kt, :],
                                             wd_t[:, g, c0_:c0_ + cw_],
                                             start=(kt == 0),
                                             stop=(kt == c.FT - 1))
                darg = ar_round(pd, "d")
                add_transposed(darg, xslim)
                continue

            # ======== non-slim: per-batch pipelined o / MLP / down ========
            # ---- o projection (+ residual into xrows), AG per batch ----
            xbo_b = [dram.tile([c.OR, c.S], bf16, tag=f"xbo{l}_{b}",
                               name=f"xbo{l}_{b}") for b in range(c.B)]
            xgo_b = [dram.tile([c.H, c.S], bf16, addr_space=SHARED,
                               tag=f"xgo{l}_{b}", name=f"xgo{l}_{b}")
                     for b in range(c.B)]
            ln2_ssg = []
            for b in range(c.B):
                og_b = [(ot * P, P, c.S, b) for ot in range(c.OT)]
                orhs = lambda kt, rk, rl: rl[:]

                def oload(kt, _b=b):
                    t = xpool.tile([P, c.S], bf16, tag="orhs",
                                   name="orhs", bufs=3)
                    nc.scalar.dma_start(
                        out=t[:],
                        in_=ctxgs[_b][kt * P:(kt + 1) * P, :])
                    return t
                ops_b = kouter_pass(c.KT, lambda k0, n: wo[l][k0:k0 + n],
                                    c.OR, og_b, orhs, rhs_load=oload,
                                    name=f"op{b}")
                for gi, (c0, cw, n, rk) in enumerate(og_b):
                    ot = c0 // P
                    xsl = xrows[:, ot, b * c.S:(b + 1) * c.S]
                    nc.vector.tensor_add(xsl, xsl, ops_b[gi][:])
                    st = xpool.tile([P, n], bf16, tag="xst", name="xst",
                                    bufs=2)
                    nc.scalar.copy(st[:], xsl)
                    nc.sync.dma_start(out=xbo_b[b][ot * P:(ot + 1) * P, :],
                                      in_=st[:])
                ln2_ssg.append(emit_sumsq_ar(b * c.S, c.S, tag=f"o{l}_{b}"))
                nc.gpsimd.collective_compute(
                    "AllGather", OP.bypass, replica_groups=RG,
                    ins=[xbo_b[b][:]], outs=[xgo_b[b][:]])

            # ---- ln2 + gated MLP, AG per batch; down chases the AGs ----
            gact = persist.tile([P, c.FT, c.S], bf16, tag="gact",
                                name=f"gact{l}")
            intb_b = [dram.tile([c.FPC, c.S], bf16, tag=f"intb{l}_{b}",
                                name=f"intb{l}_{b}") for b in range(c.B)]
            intg_b = [dram.tile([c.FF, c.S], bf16, addr_space=SHARED,
                                tag=f"intg{l}_{b}", name=f"intg{l}_{b}")
                      for b in range(c.B)]
            for b in range(c.B):
                emit_norm_post(ln2_ssg[b], xgo_b[b][:], 2 * l + 1, xn,
                               b * c.S, c.S, [(0, c.S)])
                for phase, wsrc3 in (("g", wg[l]), ("u", wu[l])):
                    gg = [(ot * P, P, c.S, ot) for ot in range(c.FT)]

                    def gsrc(k0, n, _w=wsrc3):
                        return _w[k0:k0 + n]
                    grhs = (lambda kt, rk, rl, _b=b:
                            xn[:, kt, _b * c.S:(_b + 1) * c.S])
                    gps = kouter_pass(c.KT, gsrc, c.FPC, gg, grhs,
                                      name=f"{phase}{b}")
                    for gi, (c0, cw, n, rk) in enumerate(gg):
                        ot = rk
                        if phase == "g":
                            sgt = xpool.tile([P, n], bf16, tag="sgt",
                                             name="sgt", bufs=2)
                            nc.scalar.activation(sgt[:], gps[gi][:],
                                                 AF.Sigmoid)
                            nc.vector.tensor_mul(gact[:, ot, :], gps[gi][:],
                                                 sgt[:])
                        else:
                            it = xpool.tile([P, n], bf16, tag="sgt",
                                            name="it", bufs=2)
                            nc.vector.tensor_mul(
                                it[:], gps[gi][:], gact[:, ot, :])
                            nc.sync.dma_start(
                                out=intb_b[b][ot * P:(ot + 1) * P, :],
                                in_=it[:])
                nc.gpsimd.collective_compute(
                    "AllGather", OP.bypass, replica_groups=RG,
                    ins=[intb_b[b][:]], outs=[intg_b[b][:]])

            # ---- down projection (+ residual), per batch ----
            xbd_b = [dram.tile([c.OR, c.S], bf16, tag=f"xbd{l}_{b}",
                               name=f"xbd{l}_{b}") for b in range(c.B)]
            xgd_b = [dram.tile([c.H, c.S], bf16, addr_space=SHARED,
                               tag=f"xgd{l}_{b}", name=f"xgd{l}_{b}")
                     for b in range(c.B)]
            ln1_ssg = []
            for b in range(c.B):
                dg_b = [(ot * P, P, c.S, b) for ot in range(c.OT)]
                drhs = lambda kt, rk, rl: rl[:]

                def dload(kt, _b=b):
                    t = xpool.tile([P, c.S], bf16, tag="orhs", name="drhs",
                                   bufs=3)
                    nc.scalar.dma_start(
                        out=t[:], in_=intg_b[_b][kt * P:(kt + 1) * P, :])
                    return t
                dps_b = kouter_pass(c.KTF, lambda k0, n: wd[l][k0:k0 + n],
                                    c.OR, dg_b, drhs, rhs_load=dload,
                                    name=f"dp{b}")
                for gi, (c0, cw, n, rk) in enumerate(dg_b):
                    ot = c0 // P
                    xsl = xrows[:, ot, b * c.S:(b + 1) * c.S]
                    nc.vector.tensor_add(xsl, xsl, dps_b[gi][:])
                    st = xpool.tile([P, n], bf16, tag="xst", name="xst2",
                                    bufs=2)
                    nc.scalar.copy(st[:], xsl)
                    nc.sync.dma_start(out=xbd_b[b][ot * P:(ot + 1) * P, :],
                                      in_=st[:])
                ln1_ssg.append(emit_sumsq_ar(b * c.S, c.S, tag=f"d{l}_{b}"))
                nc.gpsimd.collective_compute(
                    "AllGather", OP.bypass, replica_groups=RG,
                    ins=[xbd_b[b][:]], outs=[xgd_b[b][:]])
            x_src = xgd_b

        # ================= final norm + cls head =================
        xnf = persist.tile([P, c.KT, c.B], bf16, tag="xn2", name="xnf")
        emit_norm_slim(xslim, 2 * c.L, xnf)

        # flipped w1 pass: xnf [P,B] stationary, w1 chunks moving
        CC2 = (c.CLS + 511) // 512
        hts = [psum.tile([c.B, min(512, c.CLS - i * 512)], f32, tag="ps",
                         name=f"ht{i}") for i in range(CC2)]
        WB = max(1, min(6, c.KT))
        for blk, k0 in enumerate(range(0, c.KT, WB)):
            kn = min(WB, c.KT - k0)
            wt = wpool.tile([P, WB, c.CLS], bf16,
                            tag="wq_sb" if blk % 2 == 0 else "wk_sb",
                            name=f"w1w{blk}", bufs=1)
            nc.scalar.dma_start(
                out=wt[:, 0:kn, :],
                in_=w1t[k0:k0 + kn].rearrange("g p m -> p g m"))
            for g in range(kn):
                kt = k0 + g
                for i in range(CC2):
                    cw_ = min(512, c.CLS - i * 512)
                    nc.tensor.matmul(hts[i][:], xnf[:, kt, :],
                                     wt[:, g, i * 512:i * 512 + cw_],
                                     start=(kt == 0), stop=(kt == c.KT - 1))
        hflat = spool.tile([c.B, c.CLS], bf16, tag="sg1", name="hflat",
                           bufs=2)
        for i in range(CC2):
            cw_ = min(512, c.CLS - i * 512)
            nc.vector.tensor_copy(hflat[:, i * 512:i * 512 + cw_],
                                  hts[i][:])
        hps = []
        for ot in range(c.CT):
            tp = psum.tile([P, c.B], bf16, tag="ps", name=f"hps{ot}")
            nc.tensor.transpose(tp[:], hflat[:, ot * P:(ot + 1) * P],
                                eye2b_sb[:])
            hps.append(tp)
        h_sb = persist.tile([P, c.CT, c.B], bf16, tag="h_sb", name="h_sb")
        mn = psum.tile([1, c.B], f32, tag="ps", name="mn")
        ssq = psum.tile([1, c.B], f32, tag="ps", name="ssq")
        for ot in range(c.CT):
            nc.scalar.activation(h_sb[:, ot, :], hps[ot][:], AF.Relu,
                                 bias=b1_sb[:, ot:ot + 1])
            hq = spool.tile([P, c.B], f32, tag="hq", name="hq")
            nc.vector.tensor_mul(hq[:], h_sb[:, ot, :], h_sb[:, ot, :])
            nc.tensor.matmul(mn[:], ones_cbf[:], h_sb[:, ot, :],
                             start=(ot == 0), stop=(ot == c.CT - 1))
            nc.tensor.matmul(ssq[:], ones_c32[:], hq[:],
                             start=(ot == 0), stop=(ot == c.CT - 1))
        m_sb = spool.tile([1, c.B], f32, tag="m_sb", name="m_sb")
        nc.vector.tensor_scalar_mul(m_sb[:], mn[:], 1.0 / c.CLS)
        s_sb = spool.tile([1, c.B], f32, tag="s_sb", name="s_sb")
        nc.vector.tensor_scalar_mul(s_sb[:], ssq[:], 1.0 / c.CLS)
        msq = spool.tile([1, c.B], f32, tag="msq", name="msq")
        nc.vector.tensor_mul(msq[:], m_sb[:], m_sb[:])
        var = spool.tile([1, c.B], f32, tag="var", name="var")
        nc.vector.tensor_sub(var[:], s_sb[:], msq[:])
        lv = spool.tile([1, c.B], f32, tag="lv", name="lv")
        nc.scalar.activation(lv[:], var[:], AF.Ln, bias=eps_col[0:1, :])
        rstd = spool.tile([1, c.B], f32, tag="rstd", name="rstd")
        nc.scalar.activation(rstd[:], lv[:], AF.Exp, scale=-0.5)
        bmp = psum.tile([P, c.B], f32, tag="ps", name="bmp")
        nc.tensor.matmul(bmp[:], ones_r32[:], m_sb[:], start=True, stop=True)
        bm_sb = spool.tile([P, c.B], f32, tag="bm", name="bm")
        nc.vector.tensor_copy(bm_sb[:], bmp[:])
        brp = psum.tile([P, c.B], f32, tag="ps", name="brp")
        nc.tensor.matmul(brp[:], ones_r32[:], rstd[:], start=True, stop=True)
        br_sb = spool.tile([P, c.B], f32, tag="br", name="br")
        nc.vector.tensor_copy(br_sb[:], brp[:])

        lg = psum.tile([c.NCLS, c.B], f32, tag="ps", name="lg")
        for ot in range(c.CT):
            t1 = spool.tile([P, c.B], f32, tag="ct1", name="ct1")
            nc.vector.tensor_sub(t1[:], h_sb[:, ot, :], bm_sb[:])
            t2 = spool.tile([P, c.B], f32, tag="ct2", name="ct2")
            nc.vector.tensor_mul(t2[:], t1[:], br_sb[:])
            hn = spool.tile([P, c.B], bf16, tag="hn", name="hn")
            nc.vector.tensor_scalar(hn[:], t2[:], g_sb_c[:, ot:ot + 1],
                                    bcol_sb[:, ot:ot + 1], OP.mult, OP.add)
            w2w = wpool.tile([P, c.NCLS], bf16, tag="w2w", name="w2w")
            nc.sync.dma_start(out=w2w[:], in_=w2t[ot])
            nc.tensor.matmul(lg[:], w2w[:], hn[:],
                             start=(ot == 0), stop=(ot == c.CT - 1))
        lg_sb = spool.tile([c.NCLS, c.B], f32, tag="lg_sb", name="lg_sb")
        nc.vector.tensor_scalar(lg_sb[:], lg[:], b2_sb[:], None, OP.add)
        nc.sync.dma_start(out=out_d, in_=lg_sb[:])

    nc.compile()
    return nc


# ----------------------------------------------------------------------------
# entry point
# ----------------------------------------------------------------------------

_CACHE = {}


def _get_nc(cfg):
    if cfg not in _CACHE:
        _CACHE[cfg] = build_nc(cfg)
    return _CACHE[cfg]


def run(cfg, inputs, trace=False, **kw):
    from concourse.bass_utils import run_bass_kernel_spmd
    in_maps = host_prep(cfg, inputs)
    nc = _get_nc(cfg)
    res = run_bass_kernel_spmd(nc, in_maps, core_ids=list(range(cfg.NC)),
                               trace=trace, **kw)
    out = np.asarray(res.results[0]["logits_out"])  # [NCLS, B]
    return np.ascontiguousarray(out.T.astype(np.float32)), res


def kernel(**inputs):
    inputs = {k: np.asarray(v) for k, v in inputs.items()}
    out, _ = run(FULL_CFG, inputs)
    return out

